# revision 38
# baseline (speedup 1.0000x reference)
"""ChunkKVCompressor Trainium2 kernel.

Data-parallel over batch: core i handles batch element i (B=8 across 8 cores).
Per core:
  1. keys/values stream in 8 groups of 512 tokens. values tiles stay RESIDENT
     in SBUF (16 MB) in a quad layout (partition p holds tokens 4p..4p+3 of
     its group) so compressed values scatter straight from SBUF.
     keys load in two matching pair-layout half tiles per group.
  2. c = k + v in-place into the k tiles (DVE), rounded to float32r.
  3. Per d-block: PE transposes (4 blocks) then immediately 4 h-block
     float32r matmuls (N=512, LDWEIGHTS hidden) accumulating into 4 PSUM
     banks -- fine interleave keeps the PE HAM clock warm.
  4. relu(0.5*x + b1) on ACT; chunk sums via two DVE segment reduces
     (token order inside cT is 4*(col%128) + col//128); scores += W2.T@sums
     into one persistent PSUM row [1, 64] (mean/b2 dropped: order-preserving).
  5. On-device top-32-of-64 by rank (comparison matrix + tiny matmuls).
  6. ck: 8x 1MB indirect gathers (one row index per partition, source viewed
     as [2048 rows x 2048 elems]) staged through SBUF + contiguous writes.
     cv: 8x 2MB indirect scatters straight from the resident value tiles
     (cv viewed as [512 rows x 4096 elems]); dropped chunks get row 8192
     and are skipped via bounds_check.
"""

import sys

if "/opt/trn_rl_repo" not in sys.path:
    sys.path.insert(0, "/opt/trn_rl_repo")

import numpy as np
from contextlib import ExitStack

B, T, D, H = 8, 4096, 1024, 512
L = 64  # chunk length (tokens)
NCH = T // L  # 64 chunks
KEEP = 32  # chunks kept per batch
NG = 8  # token groups
GT = T // NG  # 512 tokens per group
NJ = D // 128  # 8 d-blocks
NHB = H // 128  # 4 h-blocks
CPG = NCH // NG  # 8 chunks per group
GCH = 4  # chunks per k-gather DMA
NDMA = KEEP // GCH  # 8 k-gather DMAs
MM_MODE = "f32r"  # "f32r" | "f32"
OOB_BIG = 8192.0  # scatter row for dropped chunks (valid rows 0..511)

# packed fp32 const tensor column layout [128, 384]
C_JLT = 0
C_TBM = 64
C_I64 = 128
C_IOTA32 = 192
C_IOTA64 = 224
C_IOTAP32 = 225
C_IOTAP16 = 226
C_B1 = 229
C_W2 = 233
C_ONES = 240  # [0:1, 240:368]
C_COLS = 384

_CACHE = {}


def _build(mode=MM_MODE):
    import concourse.bass as bass
    import concourse.tile as tile
    from concourse import bacc, mybir

    f32 = mybir.dt.float32
    i32 = mybir.dt.int32
    mm_dt = {"f32r": mybir.dt.float32r, "f32": f32}[mode]

    nc = bacc.Bacc("TRN2", target_bir_lowering=False, debug=False, num_devices=B)

    keys = nc.dram_tensor("keys", [T, D], f32, kind="ExternalInput").ap()
    values = nc.dram_tensor("values", [T, D], f32, kind="ExternalInput").ap()
    w1 = nc.dram_tensor("w1", [D, H], mm_dt, kind="ExternalInput").ap()
    ident_d = nc.dram_tensor("ident", [128, 128], mm_dt, kind="ExternalInput").ap()
    consts_d = nc.dram_tensor("consts", [128, C_COLS], f32, kind="ExternalInput").ap()
    ck = nc.dram_tensor("ck", [KEEP * L, D], f32, kind="ExternalOutput").ap()
    cv = nc.dram_tensor("cv", [KEEP * L, D], f32, kind="ExternalOutput").ap()

    def cast(ap):
        return ap.bitcast(mm_dt) if mode == "f32r" else ap

    with tile.TileContext(nc) as tc, ExitStack() as ctx:
        wp = ctx.enter_context(tc.tile_pool(name="wp", bufs=1))
        kcp = ctx.enter_context(tc.tile_pool(name="kcp", bufs=3))
        vp = ctx.enter_context(tc.tile_pool(name="vp", bufs=NG))
        ctp = ctx.enter_context(tc.tile_pool(name="ctp", bufs=3))
        htp = ctx.enter_context(tc.tile_pool(name="htp", bufs=1))
        csp = ctx.enter_context(tc.tile_pool(name="csp", bufs=8))
        selp = ctx.enter_context(tc.tile_pool(name="selp", bufs=1))
        pst_p = ctx.enter_context(tc.tile_pool(name="pst", bufs=2, space="PSUM"))
        ph_p = ctx.enter_context(tc.tile_pool(name="ph", bufs=1, space="PSUM"))
        sc_p = ctx.enter_context(tc.tile_pool(name="sc", bufs=1, space="PSUM"))
        psel = ctx.enter_context(tc.tile_pool(name="psel", bufs=1, space="PSUM"))

        # --- constants / weights to SBUF (one packed DMA + ident + w1) --
        consts = wp.tile([128, C_COLS], f32)
        nc.sync.dma_start(consts[:], consts_d[:])
        ident_sb = wp.tile([128, 128], mm_dt)
        nc.sync.dma_start(ident_sb[:], ident_d[:])

        jlt_sb = consts[:NCH, C_JLT : C_JLT + NCH]
        tbm_sb = consts[:NCH, C_TBM : C_TBM + NCH]
        i64_sb = consts[:NCH, C_I64 : C_I64 + NCH]
        iota32_sb = consts[:NCH, C_IOTA32 : C_IOTA32 + KEEP]
        iota64_sb = consts[:NCH, C_IOTA64 : C_IOTA64 + 1]
        iotap32_sb = consts[:, C_IOTAP32 : C_IOTAP32 + 1]
        iotap16_sb = consts[:, C_IOTAP16 : C_IOTAP16 + 1]
        b1sb = consts[:, C_B1 : C_B1 + NHB]
        w2sb = consts[:, C_W2 : C_W2 + NHB]
        ones_sb = consts[0:1, C_ONES : C_ONES + 128]

        # token tau = 512g + 4p + 2h + e  (k half tiles, h in {0,1})
        keys_g = keys.rearrange("(g p h e) d -> g h p e d", p=128, h=2, e=2)
        # token tau = 512g + 4p + s      (v quad tiles)
        values_g = values.rearrange("(g p s) d -> g p s d", p=128, s=4)

        # preload first group's k/v before w1 so compute starts immediately
        kt_pre = {}
        vtiles = []
        PRE = 2
        for g in range(PRE):
            for h in range(2):
                kt = kcp.tile([128, 2, D], mm_dt, tag="kc", name=f"kt{g}_{h}")
                nc.sync.dma_start(kt[:], cast(keys_g[g, h]))
                kt_pre[(g, h)] = kt
            vt = vp.tile([128, 4, D], f32, tag="v", name=f"vt{g}")
            nc.sync.dma_start(vt[:], values_g[g])
            vtiles.append(vt)

        w1sb = wp.tile([128, NJ, H], mm_dt)  # [p, j, hh]; d = j*128 + p
        nc.sync.dma_start(w1sb[:], w1.rearrange("(j p) hh -> p j hh", p=128))

        scores_ps = sc_p.tile([1, NCH], f32, space="PSUM")

        # pin the PE to the emitted transpose/matmul alternation: the
        # scheduler otherwise batches matmuls densely and lets transposes
        # trickle, which drops the PE HAM clock to 1.2 GHz half the time
        last_pe = [None]

        def pe(bi):
            if last_pe[0] is not None:
                tile.add_dep_helper(
                    bi.ins, last_pe[0], sync=False, reason="pe order"
                )
            last_pe[0] = bi.ins
            return bi

        # --- scoring ----------------------------------------------------
        pending_scores = []
        for g in range(NG):
            if g < PRE:
                khalf = [kt_pre[(g, 0)], kt_pre[(g, 1)]]
                vtile = vtiles[g]
            else:
                khalf = []
                for h in range(2):
                    kt = kcp.tile([128, 2, D], mm_dt, tag="kc", name=f"kt{g}_{h}")
                    nc.sync.dma_start(kt[:], cast(keys_g[g, h]))
                    khalf.append(kt)
                vtile = vp.tile([128, 4, D], f32, tag="v", name=f"vt{g}")
                nc.sync.dma_start(vtile[:], values_g[g])
                vtiles.append(vtile)
            # c = k + v in place (rounded to mm_dt); GPSIMD is idle during
            # scoring and keeps these big adds off the DVE queue, which
            # must serve the PSUM->SBUF copies with low latency
            for h in range(2):
                nc.gpsimd.tensor_add(
                    khalf[h][:], khalf[h][:], vtile[:, 2 * h : 2 * h + 2, :]
                )

            # cT columns: col = 128*(2h+e) + p  ->  token 4p + 2h + e
            # two half tiles (j 0-3 / 4-7) so group g+1 can start while
            # group g's second half is still being consumed
            cTa = ctp.tile([128, NJ // 2, GT], mm_dt, tag="cT", name=f"cTa{g}")
            cTb = ctp.tile([128, NJ // 2, GT], mm_dt, tag="cT", name=f"cTb{g}")

            def cT_slice(j):
                t = cTa if j < NJ // 2 else cTb
                return t[:, j % (NJ // 2), :]
            phs = [
                ph_p.tile([128, GT], f32, tag=f"ph{hb}", name=f"ph{hb}_{g}")
                for hb in range(NHB)
            ]
            def emit_mms(j):
                for hb in range(NHB):
                    pe(
                        nc.tensor.matmul(
                            phs[hb][:],
                            w1sb[:, j, 128 * hb : 128 * (hb + 1)],
                            cT_slice(j),
                            start=(j == 0),
                            stop=(j == NJ - 1),
                        )
                    )

            # transposes run one d-block ahead of the matmuls so the PE
            # never waits on the PSUM->SBUF copy of the block it multiplies;
            # the previous group's tiny score matmuls slot into the gaps
            for j in range(NJ):
                pst = pst_p.tile([128, GT], mm_dt, tag="pst")
                for h in range(2):
                    for e in range(2):
                        s = 2 * h + e
                        pe(
                            nc.tensor.transpose(
                                pst[:, 128 * s : 128 * (s + 1)],
                                khalf[h][:, e, 128 * j : 128 * (j + 1)],
                                ident_sb[:],
                            )
                        )
                nc.vector.tensor_copy(cT_slice(j), pst[:])
                if j >= 1:
                    emit_mms(j - 1)
                if 2 <= j <= 5 and pending_scores:
                    pe(pending_scores.pop(0)())
            emit_mms(NJ - 1)
            for hb in range(NHB):
                ht = htp.tile([128, GT], f32, tag="ht")
                nc.scalar.activation(
                    ht[:],
                    phs[hb][:],
                    mybir.ActivationFunctionType.Relu,
                    bias=b1sb[:, hb : hb + 1],
                    scale=0.5,
                )
                # chunk of col = (col%128)//16; reduce pp then s
                red1 = csp.tile([128, 32], f32, tag="red1")
                nc.vector.tensor_reduce(
                    red1[:],
                    ht.rearrange("p (s pg pp) -> p s pg pp", s=4, pg=8),
                    axis=mybir.AxisListType.X,
                    op=mybir.AluOpType.add,
                )
                csum = csp.tile([128, CPG], f32, tag="csum")
                nc.vector.tensor_reduce(
                    csum[:],
                    red1.rearrange("p (s pg) -> p pg s", s=4),
                    axis=mybir.AxisListType.X,
                    op=mybir.AluOpType.add,
                )

                def make_score(g=g, hb=hb, csum=csum):
                    return nc.tensor.matmul(
                        scores_ps[0:1, CPG * g : CPG * (g + 1)],
                        w2sb[:, hb : hb + 1],
                        csum[:],
                        start=(hb == 0),
                        stop=(hb == NHB - 1),
                    )

                pending_scores.append(make_score)
        while pending_scores:
            pe(pending_scores.pop(0)())

        # --- top-32 selection (all fp32, exact) ------------------------
        scores_sb = selp.tile([1, NCH], f32, tag="sel_s")
        nc.vector.tensor_copy(scores_sb[:], scores_ps[:])

        sT_ps = psel.tile([NCH, 1], f32, space="PSUM", tag="psel")
        nc.tensor.matmul(sT_ps[:], scores_sb[:], ones_sb[:, 0:1])
        sT_sb = selp.tile([NCH, 1], f32, tag="sel_sT")
        nc.vector.tensor_copy(sT_sb[:], sT_ps[:])

        r_ps = psel.tile([NCH, NCH], f32, space="PSUM", tag="psel")
        nc.tensor.matmul(r_ps[:], ones_sb[:, :NCH], scores_sb[:])
        r_sb = selp.tile([NCH, NCH], f32, tag="sel_r")
        nc.vector.tensor_copy(r_sb[:], r_ps[:])

        g_sb = selp.tile([NCH, NCH], f32, tag="sel_g")
        nc.vector.tensor_scalar(
            g_sb[:], r_sb[:], sT_sb[:], None, op0=mybir.AluOpType.is_gt
        )
        eq_sb = selp.tile([NCH, NCH], f32, tag="sel_eq")
        nc.vector.tensor_scalar(
            eq_sb[:], r_sb[:], sT_sb[:], None, op0=mybir.AluOpType.is_equal
        )
        tie_sb = selp.tile([NCH, NCH], f32, tag="sel_tie")
        nc.vector.tensor_mul(tie_sb[:], eq_sb[:], tbm_sb)
        nc.vector.tensor_add(g_sb[:], g_sb[:], tie_sb[:])
        rank_sb = selp.tile([NCH, 1], f32, tag="sel_rank")
        nc.vector.tensor_reduce(
            rank_sb[:], g_sb[:], axis=mybir.AxisListType.X, op=mybir.AluOpType.add
        )
        keep_sb = selp.tile([NCH, 1], f32, tag="sel_keep")
        nc.vector.tensor_scalar(
            keep_sb[:], rank_sb[:], float(KEEP) - 0.5, None, op0=mybir.AluOpType.is_lt
        )

        dest_ps = psel.tile([NCH, 1], f32, space="PSUM", tag="psel")
        nc.tensor.matmul(dest_ps[:], jlt_sb, keep_sb[:])
        dest_sb = selp.tile([NCH, 1], f32, tag="sel_dest")
        nc.vector.tensor_copy(dest_sb[:], dest_ps[:])

        # M[i, o] = 1 iff chunk i goes to slot o
        sel1_sb = selp.tile([NCH, KEEP], f32, tag="sel_m")
        nc.vector.tensor_scalar(
            sel1_sb[:], iota32_sb, dest_sb[:], None, op0=mybir.AluOpType.is_equal
        )
        nc.vector.tensor_scalar(
            sel1_sb[:], sel1_sb[:], keep_sb[:], None, op0=mybir.AluOpType.mult
        )

        # --- ck: gather selected key chunks ----------------------------
        # keys viewed as [2048 rows, 2048 elems]; chunk = 32 rows; each DMA
        # moves 4 chunks: partition p fetches row 32*idx[4d + p//32] + p%32.
        keys_rows = keys.rearrange("(r q) d -> r (q d)", q=2)
        ck_rows = ck.rearrange("(r q) d -> r (q d)", q=2)

        selrep = selp.tile([NCH, NDMA * 128], f32, tag="sel_rep", bufs=1)
        for d in range(NDMA):
            nc.vector.tensor_copy(
                selrep.rearrange("i (d o u) -> i d o u", d=NDMA, u=32)[:, d],
                sel1_sb[:, GCH * d : GCH * (d + 1)].to_broadcast([NCH, GCH, 32]),
            )
        rowidx = []
        for d in range(NDMA):
            rsel_ps = psel.tile([128, 1], f32, space="PSUM", tag="psel")
            nc.tensor.matmul(rsel_ps[:], selrep[:, 128 * d : 128 * (d + 1)], iota64_sb)
            rowf = selp.tile([128, 1], f32, tag="sel_rowf")
            nc.vector.tensor_scalar(
                rowf[:],
                rsel_ps[:],
                32.0,
                iotap32_sb,
                op0=mybir.AluOpType.mult,
                op1=mybir.AluOpType.add,
            )
            ridx = selp.tile([128, 1], i32, tag=f"sel_rowi{d}")
            nc.vector.tensor_copy(ridx[:], rowf[:])
            rowidx.append(ridx)

        # --- cv scatter offsets ---------------------------------------
        # cv viewed [512 rows, 4096]; chunk = 16 rows; scatter of group g
        # writes row 16*dest[8g + p//16] + p%16 (OOB_BIG if dropped).
        comb_sb = selp.tile([NCH, 1], f32, tag="sel_comb")
        nc.vector.tensor_scalar(
            comb_sb[:],
            keep_sb[:],
            -OOB_BIG,
            OOB_BIG,
            op0=mybir.AluOpType.mult,
            op1=mybir.AluOpType.add,
        )
        tmp_sb = selp.tile([NCH, 1], f32, tag="sel_tmp")
        nc.vector.tensor_scalar(
            tmp_sb[:], dest_sb[:], 16.0, None, op0=mybir.AluOpType.mult
        )
        nc.vector.tensor_add(comb_sb[:], comb_sb[:], tmp_sb[:])

        i64rep = selp.tile([NCH, NG * 128], f32, tag="sel_rep", bufs=1)
        for g in range(NG):
            nc.vector.tensor_copy(
                i64rep.rearrange("i (g o u) -> i g o u", g=NG, u=16)[:, g],
                i64_sb[:, CPG * g : CPG * (g + 1)].to_broadcast([NCH, CPG, 16]),
            )
        voffs = []
        for g in range(NG):
            voff_ps = psel.tile([128, 1], f32, space="PSUM", tag="psel")
            nc.tensor.matmul(voff_ps[:], i64rep[:, 128 * g : 128 * (g + 1)], comb_sb[:])
            vof = selp.tile([128, 1], f32, tag="sel_vof")
            nc.vector.tensor_scalar(
                vof[:], voff_ps[:], iotap16_sb, None, op0=mybir.AluOpType.add
            )
            vofi = selp.tile([128, 1], i32, tag=f"sel_vofi{g}")
            nc.vector.tensor_copy(vofi[:], vof[:])
            voffs.append(vofi)

        # --- output DMA -------------------------------------------------
        # All ck gathers issue first: the cv scatters serialize on the cv
        # WAW chain (each waits for the previous one's completion on the
        # GpSimd queue) and would otherwise stall the gather issues too.
        cv_rows = cv.rearrange("(r q) d -> r (q d)", q=4)  # [512, 4096]
        for d in range(NDMA):
            gt = kcp.tile([128, 2048], f32, tag="kc", name=f"gt{d}")
            nc.gpsimd.indirect_dma_start(
                out=gt[:],
                out_offset=None,
                in_=keys_rows,
                in_offset=bass.IndirectOffsetOnAxis(ap=rowidx[d][:], axis=0),
            )
            nc.sync.dma_start(ck_rows[128 * d : 128 * (d + 1), :], gt[:])
        for g in range(NG):
            nc.gpsimd.indirect_dma_start(
                out=cv_rows,
                out_offset=bass.IndirectOffsetOnAxis(ap=voffs[g][:], axis=0),
                in_=vtiles[g].rearrange("p s d -> p (s d)"),
                in_offset=None,
                bounds_check=KEEP * L // 4 - 1,
                oob_is_err=False,
            )

    nc.compile()
    return nc


def _host_consts(W1, b1, W2, mode=MM_MODE):
    f32 = np.float32
    c = np.zeros((128, C_COLS), dtype=f32)
    r64 = np.arange(NCH)
    c[:NCH, C_JLT : C_JLT + NCH] = r64[:, None] < r64[None, :]
    c[:NCH, C_TBM : C_TBM + NCH] = r64[None, :] < r64[:, None]
    c[:NCH, C_I64 : C_I64 + NCH] = np.eye(NCH)
    c[:NCH, C_IOTA32 : C_IOTA32 + KEEP] = np.arange(KEEP)[None, :]
    c[:NCH, C_IOTA64] = r64
    c[:, C_IOTAP32] = np.arange(128) % 32
    c[:, C_IOTAP16] = np.arange(128) % 16
    c[:, C_B1 : C_B1 + NHB] = np.asarray(b1, f32).reshape(NHB, 128).T
    c[:, C_W2 : C_W2 + NHB] = np.asarray(W2, f32)[:, 0].reshape(NHB, 128).T
    c[0, C_ONES : C_ONES + 128] = 1.0
    return {
        "w1": np.ascontiguousarray(W1, dtype=f32),
        "ident": np.eye(128, dtype=f32),
        "consts": c,
    }


def get_nc(mode=MM_MODE):
    key = ("nc", mode)
    if key not in _CACHE:
        _CACHE[key] = _build(mode)
    return _CACHE[key]


def kernel(keys, values, W1, b1, W2, b2):
    from concourse.bass_utils import run_bass_kernel_spmd

    nc = get_nc()
    keys = np.asarray(keys)
    values = np.asarray(values)
    consts = _host_consts(np.asarray(W1), np.asarray(b1), np.asarray(W2))
    in_maps = [dict(keys=keys[i], values=values[i], **consts) for i in range(B)]
    res = run_bass_kernel_spmd(nc, in_maps, list(range(B)))
    ck = np.stack([res.results[i]["ck"] for i in range(B)])
    cv = np.stack([res.results[i]["cv"] for i in range(B)])
    return ck, cv


# revision 40
# speedup vs baseline: 1.0428x; 1.0428x over previous
"""ChunkKVCompressor Trainium2 kernel.

Data-parallel over batch: core i handles batch element i (B=8 across 8 cores).
Per core:
  1. keys/values stream in 8 groups of 512 tokens. values tiles stay RESIDENT
     in SBUF (16 MB) in a quad layout (partition p holds tokens 4p..4p+3 of
     its group) so compressed values scatter straight from SBUF.
     keys load in two matching pair-layout half tiles per group.
  2. c = k + v in-place into the k tiles (DVE), rounded to float32r.
  3. Per d-block: PE transposes (4 blocks) then immediately 4 h-block
     float32r matmuls (N=512, LDWEIGHTS hidden) accumulating into 4 PSUM
     banks -- fine interleave keeps the PE HAM clock warm.
  4. relu(0.5*x + b1) on ACT; chunk sums via two DVE segment reduces
     (token order inside cT is 4*(col%128) + col//128); scores += W2.T@sums
     into one persistent PSUM row [1, 64] (mean/b2 dropped: order-preserving).
  5. On-device top-32-of-64 by rank (comparison matrix + tiny matmuls).
  6. ck: 8x 1MB indirect gathers (one row index per partition, source viewed
     as [2048 rows x 2048 elems]) staged through SBUF + contiguous writes.
     cv: 8x 2MB indirect scatters straight from the resident value tiles
     (cv viewed as [512 rows x 4096 elems]); dropped chunks get row 8192
     and are skipped via bounds_check.
"""

import sys

if "/opt/trn_rl_repo" not in sys.path:
    sys.path.insert(0, "/opt/trn_rl_repo")

import numpy as np
from contextlib import ExitStack

B, T, D, H = 8, 4096, 1024, 512
L = 64  # chunk length (tokens)
NCH = T // L  # 64 chunks
KEEP = 32  # chunks kept per batch
NG = 8  # token groups
GT = T // NG  # 512 tokens per group
NJ = D // 128  # 8 d-blocks
NHB = H // 128  # 4 h-blocks
CPG = NCH // NG  # 8 chunks per group
GCH = 4  # chunks per k-gather DMA
NDMA = KEEP // GCH  # 8 k-gather DMAs
MM_MODE = "f32r"  # "f32r" | "f32"
OOB_BIG = 8192.0  # scatter row for dropped chunks (valid rows 0..511)

# packed fp32 const tensor column layout [128, 384]
C_JLT = 0
C_TBM = 64
C_I64 = 128
C_IOTA32 = 192
C_IOTA64 = 224
C_IOTAP32 = 225
C_IOTAP16 = 226
C_B1 = 229
C_W2 = 233
C_ONES = 240  # [0:1, 240:368]
C_COLS = 384

_CACHE = {}


def _build(mode=MM_MODE):
    import concourse.bass as bass
    import concourse.tile as tile
    from concourse import bacc, mybir

    f32 = mybir.dt.float32
    i32 = mybir.dt.int32
    mm_dt = {"f32r": mybir.dt.float32r, "f32": f32}[mode]

    nc = bacc.Bacc("TRN2", target_bir_lowering=False, debug=False, num_devices=B)

    keys = nc.dram_tensor("keys", [T, D], f32, kind="ExternalInput").ap()
    values = nc.dram_tensor("values", [T, D], mm_dt, kind="ExternalInput").ap()
    w1 = nc.dram_tensor("w1", [D, H], mm_dt, kind="ExternalInput").ap()
    ident_d = nc.dram_tensor("ident", [128, 128], mm_dt, kind="ExternalInput").ap()
    consts_d = nc.dram_tensor("consts", [128, C_COLS], f32, kind="ExternalInput").ap()
    ck = nc.dram_tensor("ck", [KEEP * L, D], f32, kind="ExternalOutput").ap()
    cv = nc.dram_tensor("cv", [KEEP * L, D], f32, kind="ExternalOutput").ap()

    def cast(ap):
        return ap.bitcast(mm_dt) if mode == "f32r" else ap

    with tile.TileContext(nc) as tc, ExitStack() as ctx:
        wp = ctx.enter_context(tc.tile_pool(name="wp", bufs=1))
        kcp = ctx.enter_context(tc.tile_pool(name="kcp", bufs=4))
        vp = ctx.enter_context(tc.tile_pool(name="vp", bufs=NG))
        ctp = ctx.enter_context(tc.tile_pool(name="ctp", bufs=2))
        htp = ctx.enter_context(tc.tile_pool(name="htp", bufs=1))
        csp = ctx.enter_context(tc.tile_pool(name="csp", bufs=8))
        selp = ctx.enter_context(tc.tile_pool(name="selp", bufs=1))
        pst_p = ctx.enter_context(tc.tile_pool(name="pst", bufs=2, space="PSUM"))
        ph_p = ctx.enter_context(tc.tile_pool(name="ph", bufs=1, space="PSUM"))
        sc_p = ctx.enter_context(tc.tile_pool(name="sc", bufs=1, space="PSUM"))
        psel = ctx.enter_context(tc.tile_pool(name="psel", bufs=1, space="PSUM"))

        # --- constants / weights to SBUF (one packed DMA + ident + w1) --
        consts = wp.tile([128, C_COLS], f32)
        nc.sync.dma_start(consts[:], consts_d[:])
        ident_sb = wp.tile([128, 128], mm_dt)
        nc.sync.dma_start(ident_sb[:], ident_d[:])

        jlt_sb = consts[:NCH, C_JLT : C_JLT + NCH]
        tbm_sb = consts[:NCH, C_TBM : C_TBM + NCH]
        i64_sb = consts[:NCH, C_I64 : C_I64 + NCH]
        iota32_sb = consts[:NCH, C_IOTA32 : C_IOTA32 + KEEP]
        iota64_sb = consts[:NCH, C_IOTA64 : C_IOTA64 + 1]
        iotap32_sb = consts[:, C_IOTAP32 : C_IOTAP32 + 1]
        iotap16_sb = consts[:, C_IOTAP16 : C_IOTAP16 + 1]
        b1sb = consts[:, C_B1 : C_B1 + NHB]
        w2sb = consts[:, C_W2 : C_W2 + NHB]
        ones_sb = consts[0:1, C_ONES : C_ONES + 128]

        # token tau = 512g + 4p + 2h + e  (k half tiles, h in {0,1})
        keys_g = keys.rearrange("(g p h e) d -> g h p e d", p=128, h=2, e=2)
        # token tau = 512g + 4p + s      (v quad tiles)
        values_g = values.rearrange("(g p s) d -> g p s d", p=128, s=4)

        # preload first group's k/v before w1 so compute starts immediately
        kt_pre = {}
        vtiles = []
        PRE = 2
        for g in range(PRE):
            for h in range(2):
                kt = kcp.tile([128, 2, D], mm_dt, tag="kc", name=f"kt{g}_{h}")
                nc.sync.dma_start(kt[:], cast(keys_g[g, h]))
                kt_pre[(g, h)] = kt
            vt = vp.tile([128, 4, D], mm_dt, tag="v", name=f"vt{g}")
            nc.sync.dma_start(vt[:], values_g[g])
            vtiles.append(vt)

        w1sb = wp.tile([128, NJ, H], mm_dt)  # [p, j, hh]; d = j*128 + p
        nc.sync.dma_start(w1sb[:], w1.rearrange("(j p) hh -> p j hh", p=128))

        scores_ps = sc_p.tile([1, NCH], f32, space="PSUM")

        # pin the PE to the emitted transpose/matmul alternation: the
        # scheduler otherwise batches matmuls densely and lets transposes
        # trickle, which drops the PE HAM clock to 1.2 GHz half the time
        last_pe = [None]

        def pe(bi):
            if last_pe[0] is not None:
                tile.add_dep_helper(
                    bi.ins, last_pe[0], sync=False, reason="pe order"
                )
            last_pe[0] = bi.ins
            return bi

        # --- scoring ----------------------------------------------------
        pending_scores = []
        for g in range(NG):
            if g < PRE:
                khalf = [kt_pre[(g, 0)], kt_pre[(g, 1)]]
                vtile = vtiles[g]
            else:
                khalf = []
                for h in range(2):
                    kt = kcp.tile([128, 2, D], mm_dt, tag="kc", name=f"kt{g}_{h}")
                    nc.sync.dma_start(kt[:], cast(keys_g[g, h]))
                    khalf.append(kt)
                vtile = vp.tile([128, 4, D], mm_dt, tag="v", name=f"vt{g}")
                nc.sync.dma_start(vtile[:], values_g[g])
                vtiles.append(vtile)
            # cT columns: col = 128*(2h+e) + p  ->  token 4p + 2h + e
            # two half tiles (j 0-3 / 4-7) so group g+1 can start while
            # group g's second half is still being consumed
            cTa = ctp.tile([128, NJ // 2, GT], mm_dt, tag="cT", name=f"cTa{g}")
            cTb = ctp.tile([128, NJ // 2, GT], mm_dt, tag="cT", name=f"cTb{g}")

            def cT_slice(j):
                t = cTa if j < NJ // 2 else cTb
                return t[:, j % (NJ // 2), :]
            phs = [
                ph_p.tile([128, GT], f32, tag=f"ph{hb}", name=f"ph{hb}_{g}")
                for hb in range(NHB)
            ]
            def emit_mms(j):
                for hb in range(NHB):
                    pe(
                        nc.tensor.matmul(
                            phs[hb][:],
                            w1sb[:, j, 128 * hb : 128 * (hb + 1)],
                            cT_slice(j),
                            start=(j == 0),
                            stop=(j == NJ - 1),
                        )
                    )

            # transposes run one d-block ahead of the matmuls so the PE
            # never waits on the PSUM->SBUF copy of the block it multiplies;
            # the previous group's tiny score matmuls slot into the gaps
            for j in range(NJ):
                pst = pst_p.tile([128, GT], mm_dt, tag="pst")
                for h in range(2):
                    for e in range(2):
                        s = 2 * h + e
                        pe(
                            nc.tensor.matmul(
                                pst[:, 128 * s : 128 * (s + 1)],
                                khalf[h][:, e, 128 * j : 128 * (j + 1)],
                                ident_sb[:],
                                is_transpose=True,
                                start=True,
                                stop=False,
                            )
                        )
                        pe(
                            nc.tensor.matmul(
                                pst[:, 128 * s : 128 * (s + 1)],
                                vtile[:, s, 128 * j : 128 * (j + 1)],
                                ident_sb[:],
                                is_transpose=True,
                                start=False,
                                stop=True,
                            )
                        )
                nc.vector.tensor_copy(cT_slice(j), pst[:])
                if j >= 1:
                    emit_mms(j - 1)
                if 2 <= j <= 5 and pending_scores:
                    pe(pending_scores.pop(0)())
            emit_mms(NJ - 1)
            for hb in range(NHB):
                ht = htp.tile([128, GT], f32, tag="ht")
                nc.scalar.activation(
                    ht[:],
                    phs[hb][:],
                    mybir.ActivationFunctionType.Relu,
                    bias=b1sb[:, hb : hb + 1],
                    scale=0.5,
                )
                # chunk of col = (col%128)//16; reduce pp then s
                red1 = csp.tile([128, 32], f32, tag="red1")
                nc.vector.tensor_reduce(
                    red1[:],
                    ht.rearrange("p (s pg pp) -> p s pg pp", s=4, pg=8),
                    axis=mybir.AxisListType.X,
                    op=mybir.AluOpType.add,
                )
                csum = csp.tile([128, CPG], f32, tag="csum")
                nc.vector.tensor_reduce(
                    csum[:],
                    red1.rearrange("p (s pg) -> p pg s", s=4),
                    axis=mybir.AxisListType.X,
                    op=mybir.AluOpType.add,
                )

                def make_score(g=g, hb=hb, csum=csum):
                    return nc.tensor.matmul(
                        scores_ps[0:1, CPG * g : CPG * (g + 1)],
                        w2sb[:, hb : hb + 1],
                        csum[:],
                        start=(hb == 0),
                        stop=(hb == NHB - 1),
                    )

                pending_scores.append(make_score)
        while pending_scores:
            pe(pending_scores.pop(0)())

        # --- top-32 selection (all fp32, exact) ------------------------
        scores_sb = selp.tile([1, NCH], f32, tag="sel_s")
        nc.vector.tensor_copy(scores_sb[:], scores_ps[:])

        sT_ps = psel.tile([NCH, 1], f32, space="PSUM", tag="psel")
        nc.tensor.matmul(sT_ps[:], scores_sb[:], ones_sb[:, 0:1])
        sT_sb = selp.tile([NCH, 1], f32, tag="sel_sT")
        nc.vector.tensor_copy(sT_sb[:], sT_ps[:])

        r_ps = psel.tile([NCH, NCH], f32, space="PSUM", tag="psel")
        nc.tensor.matmul(r_ps[:], ones_sb[:, :NCH], scores_sb[:])
        r_sb = selp.tile([NCH, NCH], f32, tag="sel_r")
        nc.vector.tensor_copy(r_sb[:], r_ps[:])

        g_sb = selp.tile([NCH, NCH], f32, tag="sel_g")
        nc.vector.tensor_scalar(
            g_sb[:], r_sb[:], sT_sb[:], None, op0=mybir.AluOpType.is_gt
        )
        eq_sb = selp.tile([NCH, NCH], f32, tag="sel_eq")
        nc.vector.tensor_scalar(
            eq_sb[:], r_sb[:], sT_sb[:], None, op0=mybir.AluOpType.is_equal
        )
        tie_sb = selp.tile([NCH, NCH], f32, tag="sel_tie")
        nc.vector.tensor_mul(tie_sb[:], eq_sb[:], tbm_sb)
        nc.vector.tensor_add(g_sb[:], g_sb[:], tie_sb[:])
        rank_sb = selp.tile([NCH, 1], f32, tag="sel_rank")
        nc.vector.tensor_reduce(
            rank_sb[:], g_sb[:], axis=mybir.AxisListType.X, op=mybir.AluOpType.add
        )
        keep_sb = selp.tile([NCH, 1], f32, tag="sel_keep")
        nc.vector.tensor_scalar(
            keep_sb[:], rank_sb[:], float(KEEP) - 0.5, None, op0=mybir.AluOpType.is_lt
        )

        dest_ps = psel.tile([NCH, 1], f32, space="PSUM", tag="psel")
        nc.tensor.matmul(dest_ps[:], jlt_sb, keep_sb[:])
        dest_sb = selp.tile([NCH, 1], f32, tag="sel_dest")
        nc.vector.tensor_copy(dest_sb[:], dest_ps[:])

        # M[i, o] = 1 iff chunk i goes to slot o
        sel1_sb = selp.tile([NCH, KEEP], f32, tag="sel_m")
        nc.vector.tensor_scalar(
            sel1_sb[:], iota32_sb, dest_sb[:], None, op0=mybir.AluOpType.is_equal
        )
        nc.vector.tensor_scalar(
            sel1_sb[:], sel1_sb[:], keep_sb[:], None, op0=mybir.AluOpType.mult
        )

        # --- ck: gather selected key chunks ----------------------------
        # keys viewed as [2048 rows, 2048 elems]; chunk = 32 rows; each DMA
        # moves 4 chunks: partition p fetches row 32*idx[4d + p//32] + p%32.
        keys_rows = keys.rearrange("(r q) d -> r (q d)", q=2)
        ck_rows = ck.rearrange("(r q) d -> r (q d)", q=2)

        selrep = selp.tile([NCH, NDMA * 128], f32, tag="sel_rep", bufs=1)
        for d in range(NDMA):
            nc.vector.tensor_copy(
                selrep.rearrange("i (d o u) -> i d o u", d=NDMA, u=32)[:, d],
                sel1_sb[:, GCH * d : GCH * (d + 1)].to_broadcast([NCH, GCH, 32]),
            )
        rowidx = []
        for d in range(NDMA):
            rsel_ps = psel.tile([128, 1], f32, space="PSUM", tag="psel")
            nc.tensor.matmul(rsel_ps[:], selrep[:, 128 * d : 128 * (d + 1)], iota64_sb)
            rowf = selp.tile([128, 1], f32, tag="sel_rowf")
            nc.vector.tensor_scalar(
                rowf[:],
                rsel_ps[:],
                32.0,
                iotap32_sb,
                op0=mybir.AluOpType.mult,
                op1=mybir.AluOpType.add,
            )
            ridx = selp.tile([128, 1], i32, tag=f"sel_rowi{d}")
            nc.vector.tensor_copy(ridx[:], rowf[:])
            rowidx.append(ridx)

        # --- cv scatter offsets ---------------------------------------
        # cv viewed [512 rows, 4096]; chunk = 16 rows; scatter of group g
        # writes row 16*dest[8g + p//16] + p%16 (OOB_BIG if dropped).
        comb_sb = selp.tile([NCH, 1], f32, tag="sel_comb")
        nc.vector.tensor_scalar(
            comb_sb[:],
            keep_sb[:],
            -OOB_BIG,
            OOB_BIG,
            op0=mybir.AluOpType.mult,
            op1=mybir.AluOpType.add,
        )
        tmp_sb = selp.tile([NCH, 1], f32, tag="sel_tmp")
        nc.vector.tensor_scalar(
            tmp_sb[:], dest_sb[:], 16.0, None, op0=mybir.AluOpType.mult
        )
        nc.vector.tensor_add(comb_sb[:], comb_sb[:], tmp_sb[:])

        i64rep = selp.tile([NCH, NG * 128], f32, tag="sel_rep", bufs=1)
        for g in range(NG):
            nc.vector.tensor_copy(
                i64rep.rearrange("i (g o u) -> i g o u", g=NG, u=16)[:, g],
                i64_sb[:, CPG * g : CPG * (g + 1)].to_broadcast([NCH, CPG, 16]),
            )
        voffs = []
        for g in range(NG):
            voff_ps = psel.tile([128, 1], f32, space="PSUM", tag="psel")
            nc.tensor.matmul(voff_ps[:], i64rep[:, 128 * g : 128 * (g + 1)], comb_sb[:])
            vof = selp.tile([128, 1], f32, tag="sel_vof")
            nc.vector.tensor_scalar(
                vof[:], voff_ps[:], iotap16_sb, None, op0=mybir.AluOpType.add
            )
            vofi = selp.tile([128, 1], i32, tag=f"sel_vofi{g}")
            nc.vector.tensor_copy(vofi[:], vof[:])
            voffs.append(vofi)

        # --- output DMA -------------------------------------------------
        # All ck gathers issue first: the cv scatters serialize on the cv
        # WAW chain (each waits for the previous one's completion on the
        # GpSimd queue) and would otherwise stall the gather issues too.
        cv_rows = cv.rearrange("(r q) d -> r (q d)", q=4)  # [512, 4096]
        for d in range(NDMA):
            gt = kcp.tile([128, 2048], f32, tag="kc", name=f"gt{d}")
            nc.gpsimd.indirect_dma_start(
                out=gt[:],
                out_offset=None,
                in_=keys_rows,
                in_offset=bass.IndirectOffsetOnAxis(ap=rowidx[d][:], axis=0),
            )
            nc.sync.dma_start(ck_rows[128 * d : 128 * (d + 1), :], gt[:])
        for g in range(NG):
            nc.gpsimd.indirect_dma_start(
                out=cv_rows.bitcast(mm_dt),
                out_offset=bass.IndirectOffsetOnAxis(ap=voffs[g][:], axis=0),
                in_=vtiles[g].rearrange("p s d -> p (s d)"),
                in_offset=None,
                bounds_check=KEEP * L // 4 - 1,
                oob_is_err=False,
            )

    nc.compile()
    return nc


def _host_consts(W1, b1, W2, mode=MM_MODE):
    f32 = np.float32
    c = np.zeros((128, C_COLS), dtype=f32)
    r64 = np.arange(NCH)
    c[:NCH, C_JLT : C_JLT + NCH] = r64[:, None] < r64[None, :]
    c[:NCH, C_TBM : C_TBM + NCH] = r64[None, :] < r64[:, None]
    c[:NCH, C_I64 : C_I64 + NCH] = np.eye(NCH)
    c[:NCH, C_IOTA32 : C_IOTA32 + KEEP] = np.arange(KEEP)[None, :]
    c[:NCH, C_IOTA64] = r64
    c[:, C_IOTAP32] = np.arange(128) % 32
    c[:, C_IOTAP16] = np.arange(128) % 16
    c[:, C_B1 : C_B1 + NHB] = np.asarray(b1, f32).reshape(NHB, 128).T
    c[:, C_W2 : C_W2 + NHB] = np.asarray(W2, f32)[:, 0].reshape(NHB, 128).T
    c[0, C_ONES : C_ONES + 128] = 1.0
    return {
        "w1": np.ascontiguousarray(W1, dtype=f32),
        "ident": np.eye(128, dtype=f32),
        "consts": c,
    }


def get_nc(mode=MM_MODE):
    key = ("nc", mode)
    if key not in _CACHE:
        _CACHE[key] = _build(mode)
    return _CACHE[key]


def kernel(keys, values, W1, b1, W2, b2):
    from concourse.bass_utils import run_bass_kernel_spmd

    nc = get_nc()
    keys = np.asarray(keys)
    values = np.asarray(values)
    consts = _host_consts(np.asarray(W1), np.asarray(b1), np.asarray(W2))
    in_maps = [dict(keys=keys[i], values=values[i], **consts) for i in range(B)]
    res = run_bass_kernel_spmd(nc, in_maps, list(range(B)))
    ck = np.stack([res.results[i]["ck"] for i in range(B)])
    cv = np.stack([res.results[i]["cv"] for i in range(B)])
    return ck, cv


# revision 42
# speedup vs baseline: 1.0506x; 1.0075x over previous
"""ChunkKVCompressor Trainium2 kernel.

Data-parallel over batch: core i handles batch element i (B=8 across 8 cores).
Per core:
  1. keys/values stream in 8 groups of 512 tokens. values tiles stay RESIDENT
     in SBUF (16 MB) in a quad layout (partition p holds tokens 4p..4p+3 of
     its group) so compressed values scatter straight from SBUF.
     keys load in two matching pair-layout half tiles per group.
  2. c = k + v in-place into the k tiles (DVE), rounded to float32r.
  3. Per d-block: PE transposes (4 blocks) then immediately 4 h-block
     float32r matmuls (N=512, LDWEIGHTS hidden) accumulating into 4 PSUM
     banks -- fine interleave keeps the PE HAM clock warm.
  4. relu(0.5*x + b1) on ACT; chunk sums via two DVE segment reduces
     (token order inside cT is 4*(col%128) + col//128); scores += W2.T@sums
     into one persistent PSUM row [1, 64] (mean/b2 dropped: order-preserving).
  5. On-device top-32-of-64 by rank (comparison matrix + tiny matmuls).
  6. ck: 8x 1MB indirect gathers (one row index per partition, source viewed
     as [2048 rows x 2048 elems]) staged through SBUF + contiguous writes.
     cv: 8x 2MB indirect scatters straight from the resident value tiles
     (cv viewed as [512 rows x 4096 elems]); dropped chunks get row 8192
     and are skipped via bounds_check.
"""

import sys

if "/opt/trn_rl_repo" not in sys.path:
    sys.path.insert(0, "/opt/trn_rl_repo")

import numpy as np
from contextlib import ExitStack

B, T, D, H = 8, 4096, 1024, 512
L = 64  # chunk length (tokens)
NCH = T // L  # 64 chunks
KEEP = 32  # chunks kept per batch
NG = 8  # token groups
GT = T // NG  # 512 tokens per group
NJ = D // 128  # 8 d-blocks
NHB = H // 128  # 4 h-blocks
CPG = NCH // NG  # 8 chunks per group
GCH = 4  # chunks per k-gather DMA
NDMA = KEEP // GCH  # 8 k-gather DMAs
MM_MODE = "f32r"  # "f32r" | "f32"
OOB_BIG = 8192.0  # scatter row for dropped chunks (valid rows 0..511)

# packed fp32 const tensor column layout [128, 384]
C_JLT = 0
C_TBM = 64
C_I64 = 128
C_IOTA32 = 192
C_IOTA64 = 224
C_IOTAP32 = 225
C_IOTAP16 = 226
C_B1 = 229
C_W2 = 233
C_ONES = 240  # [0:1, 240:368]
C_COLS = 384

_CACHE = {}


def _build(mode=MM_MODE):
    import concourse.bass as bass
    import concourse.tile as tile
    from concourse import bacc, mybir

    f32 = mybir.dt.float32
    i32 = mybir.dt.int32
    mm_dt = {"f32r": mybir.dt.float32r, "f32": f32}[mode]

    nc = bacc.Bacc("TRN2", target_bir_lowering=False, debug=False, num_devices=B)

    keys = nc.dram_tensor("keys", [T, D], f32, kind="ExternalInput").ap()
    values = nc.dram_tensor("values", [T, D], f32, kind="ExternalInput").ap()
    w1 = nc.dram_tensor("w1", [D, H], mm_dt, kind="ExternalInput").ap()
    ident_d = nc.dram_tensor("ident", [128, 128], f32, kind="ExternalInput").ap()
    consts_d = nc.dram_tensor("consts", [128, C_COLS], f32, kind="ExternalInput").ap()
    ck = nc.dram_tensor("ck", [KEEP * L, D], f32, kind="ExternalOutput").ap()
    cv = nc.dram_tensor("cv", [KEEP * L, D], f32, kind="ExternalOutput").ap()

    def cast(ap):
        return ap.bitcast(mm_dt) if mode == "f32r" else ap

    with tile.TileContext(nc) as tc, ExitStack() as ctx:
        wp = ctx.enter_context(tc.tile_pool(name="wp", bufs=1))
        kcp = ctx.enter_context(tc.tile_pool(name="kcp", bufs=4))
        vp = ctx.enter_context(tc.tile_pool(name="vp", bufs=NG))
        ctp = ctx.enter_context(tc.tile_pool(name="ctp", bufs=2))
        htp = ctx.enter_context(tc.tile_pool(name="htp", bufs=1))
        csp = ctx.enter_context(tc.tile_pool(name="csp", bufs=8))
        selp = ctx.enter_context(tc.tile_pool(name="selp", bufs=1))
        pst_p = ctx.enter_context(tc.tile_pool(name="pst", bufs=2, space="PSUM"))
        ph_p = ctx.enter_context(tc.tile_pool(name="ph", bufs=1, space="PSUM"))
        sc_p = ctx.enter_context(tc.tile_pool(name="sc", bufs=1, space="PSUM"))
        psel = ctx.enter_context(tc.tile_pool(name="psel", bufs=1, space="PSUM"))

        # --- constants / weights to SBUF (one packed DMA + ident + w1) --
        consts = wp.tile([128, C_COLS], f32)
        nc.sync.dma_start(consts[:], consts_d[:])
        ident_f32 = wp.tile([128, 128], f32)
        nc.sync.dma_start(ident_f32[:], ident_d[:])

        jlt_sb = consts[:NCH, C_JLT : C_JLT + NCH]
        tbm_sb = consts[:NCH, C_TBM : C_TBM + NCH]
        i64_sb = consts[:NCH, C_I64 : C_I64 + NCH]
        iota32_sb = consts[:NCH, C_IOTA32 : C_IOTA32 + KEEP]
        iota64_sb = consts[:NCH, C_IOTA64 : C_IOTA64 + 1]
        iotap32_sb = consts[:, C_IOTAP32 : C_IOTAP32 + 1]
        iotap16_sb = consts[:, C_IOTAP16 : C_IOTAP16 + 1]
        b1sb = consts[:, C_B1 : C_B1 + NHB]
        w2sb = consts[:, C_W2 : C_W2 + NHB]
        ones_sb = consts[0:1, C_ONES : C_ONES + 128]

        # token tau = 512g + 4p + 2h + e  (k half tiles, h in {0,1})
        keys_g = keys.rearrange("(g p h e) d -> g h p e d", p=128, h=2, e=2)
        # token tau = 512g + 4p + s      (v quad tiles)
        values_g = values.rearrange("(g p s) d -> g p s d", p=128, s=4)

        # preload first group's k/v before w1 so compute starts immediately
        kt_pre = {}
        vtiles = []
        PRE = 2
        for g in range(PRE):
            for h in range(2):
                kt = kcp.tile([128, 2, D], f32, tag="kc", name=f"kt{g}_{h}")
                nc.sync.dma_start(kt[:], keys_g[g, h])
                kt_pre[(g, h)] = kt
            vt = vp.tile([128, 4, D], f32, tag="v", name=f"vt{g}")
            nc.sync.dma_start(vt[:], values_g[g])
            vtiles.append(vt)

        w1sb = wp.tile([128, NJ, H], mm_dt)  # [p, j, hh]; d = j*128 + p
        nc.sync.dma_start(w1sb[:], w1.rearrange("(j p) hh -> p j hh", p=128))

        scores_ps = sc_p.tile([1, NCH], f32, space="PSUM")

        # pin the PE to the emitted transpose/matmul alternation: the
        # scheduler otherwise batches matmuls densely and lets transposes
        # trickle, which drops the PE HAM clock to 1.2 GHz half the time
        last_pe = [None]

        def pe(bi):
            if last_pe[0] is not None:
                tile.add_dep_helper(
                    bi.ins, last_pe[0], sync=False, reason="pe order"
                )
            last_pe[0] = bi.ins
            return bi

        # --- scoring ----------------------------------------------------
        pending_scores = []
        for g in range(NG):
            if g < PRE:
                khalf = [kt_pre[(g, 0)], kt_pre[(g, 1)]]
                vtile = vtiles[g]
            else:
                khalf = []
                for h in range(2):
                    kt = kcp.tile([128, 2, D], f32, tag="kc", name=f"kt{g}_{h}")
                    nc.sync.dma_start(kt[:], keys_g[g, h])
                    khalf.append(kt)
                vtile = vp.tile([128, 4, D], f32, tag="v", name=f"vt{g}")
                nc.sync.dma_start(vtile[:], values_g[g])
                vtiles.append(vtile)
            # cT columns: col = 128*(2h+e) + p  ->  token 4p + 2h + e
            # two half tiles (j 0-3 / 4-7) so group g+1 can start while
            # group g's second half is still being consumed
            cTa = ctp.tile([128, NJ // 2, GT], mm_dt, tag="cT", name=f"cTa{g}")
            cTb = ctp.tile([128, NJ // 2, GT], mm_dt, tag="cT", name=f"cTb{g}")

            def cT_slice(j):
                t = cTa if j < NJ // 2 else cTb
                return t[:, j % (NJ // 2), :]
            phs = [
                ph_p.tile([128, GT], f32, tag=f"ph{hb}", name=f"ph{hb}_{g}")
                for hb in range(NHB)
            ]
            def emit_mms(j):
                for hb in range(NHB):
                    pe(
                        nc.tensor.matmul(
                            phs[hb][:],
                            w1sb[:, j, 128 * hb : 128 * (hb + 1)],
                            cT_slice(j),
                            start=(j == 0),
                            stop=(j == NJ - 1),
                        )
                    )

            # transposes run one d-block ahead of the matmuls so the PE
            # never waits on the PSUM->SBUF copy of the block it multiplies;
            # the previous group's tiny score matmuls slot into the gaps
            for j in range(NJ):
                pst = pst_p.tile([128, GT], f32, tag="pst")
                for h in range(2):
                    for e in range(2):
                        s = 2 * h + e
                        pe(
                            nc.tensor.matmul(
                                pst[:, 128 * s : 128 * (s + 1)],
                                khalf[h][:, e, 128 * j : 128 * (j + 1)],
                                ident_f32[:],
                                is_transpose=True,
                                start=True,
                                stop=False,
                            )
                        )
                        pe(
                            nc.tensor.matmul(
                                pst[:, 128 * s : 128 * (s + 1)],
                                vtile[:, s, 128 * j : 128 * (j + 1)],
                                ident_f32[:],
                                is_transpose=True,
                                start=False,
                                stop=True,
                            )
                        )
                nc.vector.tensor_copy(cT_slice(j), pst[:])
                if j >= 1:
                    emit_mms(j - 1)
                if 2 <= j <= 5 and pending_scores:
                    pe(pending_scores.pop(0)())
            emit_mms(NJ - 1)
            for hb in range(NHB):
                ht = htp.tile([128, GT], f32, tag="ht")
                nc.scalar.activation(
                    ht[:],
                    phs[hb][:],
                    mybir.ActivationFunctionType.Relu,
                    bias=b1sb[:, hb : hb + 1],
                    scale=0.5,
                )
                # chunk of col = (col%128)//16; reduce pp then s
                red1 = csp.tile([128, 32], f32, tag="red1")
                nc.vector.tensor_reduce(
                    red1[:],
                    ht.rearrange("p (s pg pp) -> p s pg pp", s=4, pg=8),
                    axis=mybir.AxisListType.X,
                    op=mybir.AluOpType.add,
                )
                csum = csp.tile([128, CPG], f32, tag="csum")
                nc.vector.tensor_reduce(
                    csum[:],
                    red1.rearrange("p (s pg) -> p pg s", s=4),
                    axis=mybir.AxisListType.X,
                    op=mybir.AluOpType.add,
                )

                def make_score(g=g, hb=hb, csum=csum):
                    return nc.tensor.matmul(
                        scores_ps[0:1, CPG * g : CPG * (g + 1)],
                        w2sb[:, hb : hb + 1],
                        csum[:],
                        start=(hb == 0),
                        stop=(hb == NHB - 1),
                    )

                pending_scores.append(make_score)
        while pending_scores:
            pe(pending_scores.pop(0)())

        # --- top-32 selection (all fp32, exact) ------------------------
        scores_sb = selp.tile([1, NCH], f32, tag="sel_s")
        nc.vector.tensor_copy(scores_sb[:], scores_ps[:])

        sT_ps = psel.tile([NCH, 1], f32, space="PSUM", tag="psel")
        nc.tensor.matmul(sT_ps[:], scores_sb[:], ones_sb[:, 0:1])
        sT_sb = selp.tile([NCH, 1], f32, tag="sel_sT")
        nc.vector.tensor_copy(sT_sb[:], sT_ps[:])

        r_ps = psel.tile([NCH, NCH], f32, space="PSUM", tag="psel")
        nc.tensor.matmul(r_ps[:], ones_sb[:, :NCH], scores_sb[:])
        r_sb = selp.tile([NCH, NCH], f32, tag="sel_r")
        nc.vector.tensor_copy(r_sb[:], r_ps[:])

        g_sb = selp.tile([NCH, NCH], f32, tag="sel_g")
        nc.vector.tensor_scalar(
            g_sb[:], r_sb[:], sT_sb[:], None, op0=mybir.AluOpType.is_gt
        )
        eq_sb = selp.tile([NCH, NCH], f32, tag="sel_eq")
        nc.vector.tensor_scalar(
            eq_sb[:], r_sb[:], sT_sb[:], None, op0=mybir.AluOpType.is_equal
        )
        tie_sb = selp.tile([NCH, NCH], f32, tag="sel_tie")
        nc.vector.tensor_mul(tie_sb[:], eq_sb[:], tbm_sb)
        nc.vector.tensor_add(g_sb[:], g_sb[:], tie_sb[:])
        rank_sb = selp.tile([NCH, 1], f32, tag="sel_rank")
        nc.vector.tensor_reduce(
            rank_sb[:], g_sb[:], axis=mybir.AxisListType.X, op=mybir.AluOpType.add
        )
        keep_sb = selp.tile([NCH, 1], f32, tag="sel_keep")
        nc.vector.tensor_scalar(
            keep_sb[:], rank_sb[:], float(KEEP) - 0.5, None, op0=mybir.AluOpType.is_lt
        )

        dest_ps = psel.tile([NCH, 1], f32, space="PSUM", tag="psel")
        nc.tensor.matmul(dest_ps[:], jlt_sb, keep_sb[:])
        dest_sb = selp.tile([NCH, 1], f32, tag="sel_dest")
        nc.vector.tensor_copy(dest_sb[:], dest_ps[:])

        # M[i, o] = 1 iff chunk i goes to slot o
        sel1_sb = selp.tile([NCH, KEEP], f32, tag="sel_m")
        nc.vector.tensor_scalar(
            sel1_sb[:], iota32_sb, dest_sb[:], None, op0=mybir.AluOpType.is_equal
        )
        nc.vector.tensor_scalar(
            sel1_sb[:], sel1_sb[:], keep_sb[:], None, op0=mybir.AluOpType.mult
        )

        # --- ck: gather selected key chunks ----------------------------
        # keys viewed as [2048 rows, 2048 elems]; chunk = 32 rows; each DMA
        # moves 4 chunks: partition p fetches row 32*idx[4d + p//32] + p%32.
        keys_rows = keys.rearrange("(r q) d -> r (q d)", q=2)
        ck_rows = ck.rearrange("(r q) d -> r (q d)", q=2)

        selrep = selp.tile([NCH, NDMA * 128], f32, tag="sel_rep", bufs=1)
        for d in range(NDMA):
            nc.vector.tensor_copy(
                selrep.rearrange("i (d o u) -> i d o u", d=NDMA, u=32)[:, d],
                sel1_sb[:, GCH * d : GCH * (d + 1)].to_broadcast([NCH, GCH, 32]),
            )
        rowidx = []
        for d in range(NDMA):
            rsel_ps = psel.tile([128, 1], f32, space="PSUM", tag="psel")
            nc.tensor.matmul(rsel_ps[:], selrep[:, 128 * d : 128 * (d + 1)], iota64_sb)
            rowf = selp.tile([128, 1], f32, tag="sel_rowf")
            nc.vector.tensor_scalar(
                rowf[:],
                rsel_ps[:],
                32.0,
                iotap32_sb,
                op0=mybir.AluOpType.mult,
                op1=mybir.AluOpType.add,
            )
            ridx = selp.tile([128, 1], i32, tag=f"sel_rowi{d}")
            nc.vector.tensor_copy(ridx[:], rowf[:])
            rowidx.append(ridx)

        # --- cv scatter offsets ---------------------------------------
        # cv viewed [512 rows, 4096]; chunk = 16 rows; scatter of group g
        # writes row 16*dest[8g + p//16] + p%16 (OOB_BIG if dropped).
        comb_sb = selp.tile([NCH, 1], f32, tag="sel_comb")
        nc.vector.tensor_scalar(
            comb_sb[:],
            keep_sb[:],
            -OOB_BIG,
            OOB_BIG,
            op0=mybir.AluOpType.mult,
            op1=mybir.AluOpType.add,
        )
        tmp_sb = selp.tile([NCH, 1], f32, tag="sel_tmp")
        nc.vector.tensor_scalar(
            tmp_sb[:], dest_sb[:], 16.0, None, op0=mybir.AluOpType.mult
        )
        nc.vector.tensor_add(comb_sb[:], comb_sb[:], tmp_sb[:])

        i64rep = selp.tile([NCH, NG * 128], f32, tag="sel_rep", bufs=1)
        for g in range(NG):
            nc.vector.tensor_copy(
                i64rep.rearrange("i (g o u) -> i g o u", g=NG, u=16)[:, g],
                i64_sb[:, CPG * g : CPG * (g + 1)].to_broadcast([NCH, CPG, 16]),
            )
        voffs = []
        for g in range(NG):
            voff_ps = psel.tile([128, 1], f32, space="PSUM", tag="psel")
            nc.tensor.matmul(voff_ps[:], i64rep[:, 128 * g : 128 * (g + 1)], comb_sb[:])
            vof = selp.tile([128, 1], f32, tag="sel_vof")
            nc.vector.tensor_scalar(
                vof[:], voff_ps[:], iotap16_sb, None, op0=mybir.AluOpType.add
            )
            vofi = selp.tile([128, 1], i32, tag=f"sel_vofi{g}")
            nc.vector.tensor_copy(vofi[:], vof[:])
            voffs.append(vofi)

        # --- output DMA -------------------------------------------------
        # All ck gathers issue first: the cv scatters serialize on the cv
        # WAW chain (each waits for the previous one's completion on the
        # GpSimd queue) and would otherwise stall the gather issues too.
        cv_rows = cv.rearrange("(r q) d -> r (q d)", q=4)  # [512, 4096]
        for d in range(NDMA):
            gt = kcp.tile([128, 2048], f32, tag="kc", name=f"gt{d}")
            nc.gpsimd.indirect_dma_start(
                out=gt[:],
                out_offset=None,
                in_=keys_rows,
                in_offset=bass.IndirectOffsetOnAxis(ap=rowidx[d][:], axis=0),
            )
            nc.sync.dma_start(ck_rows[128 * d : 128 * (d + 1), :], gt[:])
        for g in range(NG):
            nc.gpsimd.indirect_dma_start(
                out=cv_rows,
                out_offset=bass.IndirectOffsetOnAxis(ap=voffs[g][:], axis=0),
                in_=vtiles[g].rearrange("p s d -> p (s d)"),
                in_offset=None,
                bounds_check=KEEP * L // 4 - 1,
                oob_is_err=False,
            )

    nc.compile()
    return nc


def _host_consts(W1, b1, W2, mode=MM_MODE):
    f32 = np.float32
    c = np.zeros((128, C_COLS), dtype=f32)
    r64 = np.arange(NCH)
    c[:NCH, C_JLT : C_JLT + NCH] = r64[:, None] < r64[None, :]
    c[:NCH, C_TBM : C_TBM + NCH] = r64[None, :] < r64[:, None]
    c[:NCH, C_I64 : C_I64 + NCH] = np.eye(NCH)
    c[:NCH, C_IOTA32 : C_IOTA32 + KEEP] = np.arange(KEEP)[None, :]
    c[:NCH, C_IOTA64] = r64
    c[:, C_IOTAP32] = np.arange(128) % 32
    c[:, C_IOTAP16] = np.arange(128) % 16
    c[:, C_B1 : C_B1 + NHB] = np.asarray(b1, f32).reshape(NHB, 128).T
    c[:, C_W2 : C_W2 + NHB] = np.asarray(W2, f32)[:, 0].reshape(NHB, 128).T
    c[0, C_ONES : C_ONES + 128] = 1.0
    return {
        "w1": np.ascontiguousarray(W1, dtype=f32),
        "ident": np.eye(128, dtype=f32),
        "consts": c,
    }


def get_nc(mode=MM_MODE):
    key = ("nc", mode)
    if key not in _CACHE:
        _CACHE[key] = _build(mode)
    return _CACHE[key]


def kernel(keys, values, W1, b1, W2, b2):
    from concourse.bass_utils import run_bass_kernel_spmd

    nc = get_nc()
    keys = np.asarray(keys)
    values = np.asarray(values)
    consts = _host_consts(np.asarray(W1), np.asarray(b1), np.asarray(W2))
    in_maps = [dict(keys=keys[i], values=values[i], **consts) for i in range(B)]
    res = run_bass_kernel_spmd(nc, in_maps, list(range(B)))
    ck = np.stack([res.results[i]["ck"] for i in range(B)])
    cv = np.stack([res.results[i]["cv"] for i in range(B)])
    return ck, cv


# revision 43
# speedup vs baseline: 1.1065x; 1.0532x over previous
"""ChunkKVCompressor Trainium2 kernel.

Data-parallel over batch: core i handles batch element i (B=8 across 8 cores).
Per core:
  1. keys/values stream in 8 groups of 512 tokens. values tiles stay RESIDENT
     in SBUF (16 MB) in a quad layout (partition p holds tokens 4p..4p+3 of
     its group) so compressed values scatter straight from SBUF.
     keys load in two matching pair-layout half tiles per group.
  2. c = k + v in-place into the k tiles (DVE), rounded to float32r.
  3. Per d-block: PE transposes (4 blocks) then immediately 4 h-block
     float32r matmuls (N=512, LDWEIGHTS hidden) accumulating into 4 PSUM
     banks -- fine interleave keeps the PE HAM clock warm.
  4. relu(0.5*x + b1) on ACT; chunk sums via two DVE segment reduces
     (token order inside cT is 4*(col%128) + col//128); scores += W2.T@sums
     into one persistent PSUM row [1, 64] (mean/b2 dropped: order-preserving).
  5. On-device top-32-of-64 by rank (comparison matrix + tiny matmuls).
  6. ck: 8x 1MB indirect gathers (one row index per partition, source viewed
     as [2048 rows x 2048 elems]) staged through SBUF + contiguous writes.
     cv: 8x 2MB indirect scatters straight from the resident value tiles
     (cv viewed as [512 rows x 4096 elems]); dropped chunks get row 8192
     and are skipped via bounds_check.
"""

import sys

if "/opt/trn_rl_repo" not in sys.path:
    sys.path.insert(0, "/opt/trn_rl_repo")

import numpy as np
from contextlib import ExitStack

B, T, D, H = 8, 4096, 1024, 512
L = 64  # chunk length (tokens)
NCH = T // L  # 64 chunks
KEEP = 32  # chunks kept per batch
NG = 8  # token groups
GT = T // NG  # 512 tokens per group
NJ = D // 128  # 8 d-blocks
NHB = H // 128  # 4 h-blocks
CPG = NCH // NG  # 8 chunks per group
GCH = 4  # chunks per k-gather DMA
NDMA = KEEP // GCH  # 8 k-gather DMAs
MM_MODE = "f32r"  # "f32r" | "f32"
OOB_BIG = 8192.0  # scatter row for dropped chunks (valid rows 0..511)

# packed fp32 const tensor column layout [128, 384]
C_JLT = 0
C_TBM = 64
C_I64 = 128
C_IOTA32 = 192
C_IOTA64 = 224
C_IOTAP32 = 225
C_IOTAP16 = 226
C_B1 = 229
C_W2 = 233
C_ONES = 240  # [0:1, 240:368]
C_COLS = 384

_CACHE = {}


def _build(mode=MM_MODE):
    import concourse.bass as bass
    import concourse.tile as tile
    from concourse import bacc, mybir

    f32 = mybir.dt.float32
    i32 = mybir.dt.int32
    mm_dt = {"f32r": mybir.dt.float32r, "f32": f32}[mode]

    nc = bacc.Bacc("TRN2", target_bir_lowering=False, debug=False, num_devices=B)

    keys = nc.dram_tensor("keys", [T, D], f32, kind="ExternalInput").ap()
    values = nc.dram_tensor("values", [T, D], f32, kind="ExternalInput").ap()
    w1 = nc.dram_tensor("w1", [D, H], mm_dt, kind="ExternalInput").ap()
    ident_d = nc.dram_tensor("ident", [128, 128], f32, kind="ExternalInput").ap()
    consts_d = nc.dram_tensor("consts", [128, C_COLS], f32, kind="ExternalInput").ap()
    ck = nc.dram_tensor("ck", [KEEP * L, D], f32, kind="ExternalOutput").ap()
    cv = nc.dram_tensor("cv", [KEEP * L, D], f32, kind="ExternalOutput").ap()

    def cast(ap):
        return ap.bitcast(mm_dt) if mode == "f32r" else ap

    with tile.TileContext(nc) as tc, ExitStack() as ctx:
        wp = ctx.enter_context(tc.tile_pool(name="wp", bufs=1))
        kcp = ctx.enter_context(tc.tile_pool(name="kcp", bufs=4))
        vp = ctx.enter_context(tc.tile_pool(name="vp", bufs=NG))
        ctp = ctx.enter_context(tc.tile_pool(name="ctp", bufs=2))
        htp = ctx.enter_context(tc.tile_pool(name="htp", bufs=1))
        csp = ctx.enter_context(tc.tile_pool(name="csp", bufs=8))
        selp = ctx.enter_context(tc.tile_pool(name="selp", bufs=1))
        pst_p = ctx.enter_context(tc.tile_pool(name="pst", bufs=2, space="PSUM"))
        ph_p = ctx.enter_context(tc.tile_pool(name="ph", bufs=1, space="PSUM"))
        sc_p = ctx.enter_context(tc.tile_pool(name="sc", bufs=1, space="PSUM"))
        psel = ctx.enter_context(tc.tile_pool(name="psel", bufs=1, space="PSUM"))

        # --- constants / weights to SBUF (one packed DMA + ident + w1) --
        consts = wp.tile([128, C_COLS], f32)
        nc.sync.dma_start(consts[:], consts_d[:])
        ident_f32 = wp.tile([128, 128], f32)
        nc.sync.dma_start(ident_f32[:], ident_d[:])

        jlt_sb = consts[:NCH, C_JLT : C_JLT + NCH]
        tbm_sb = consts[:NCH, C_TBM : C_TBM + NCH]
        i64_sb = consts[:NCH, C_I64 : C_I64 + NCH]
        iota32_sb = consts[:NCH, C_IOTA32 : C_IOTA32 + KEEP]
        iota64_sb = consts[:NCH, C_IOTA64 : C_IOTA64 + 1]
        iotap32_sb = consts[:, C_IOTAP32 : C_IOTAP32 + 1]
        iotap16_sb = consts[:, C_IOTAP16 : C_IOTAP16 + 1]
        b1sb = consts[:, C_B1 : C_B1 + NHB]
        w2sb = consts[:, C_W2 : C_W2 + NHB]
        ones_sb = consts[0:1, C_ONES : C_ONES + 128]

        # token tau = 512g + 4p + 2h + e  (k half tiles, h in {0,1})
        keys_g = keys.rearrange("(g p h e) d -> g h p e d", p=128, h=2, e=2)
        # token tau = 512g + 4p + s      (v quad tiles)
        values_g = values.rearrange("(g p s) d -> g p s d", p=128, s=4)

        # preload first group's k/v before w1 so compute starts immediately
        kt_pre = {}
        vtiles = []
        PRE = 2
        for g in range(PRE):
            for h in range(2):
                kt = kcp.tile([128, 2, D], f32, tag="kc", name=f"kt{g}_{h}")
                nc.sync.dma_start(kt[:], keys_g[g, h])
                kt_pre[(g, h)] = kt
            vt = vp.tile([128, 4, D], f32, tag="v", name=f"vt{g}")
            nc.sync.dma_start(vt[:], values_g[g])
            vtiles.append(vt)

        w1sb = wp.tile([128, NJ, H], mm_dt)  # [p, j, hh]; d = j*128 + p
        nc.sync.dma_start(w1sb[:], w1.rearrange("(j p) hh -> p j hh", p=128))

        scores_ps = sc_p.tile([1, NCH], f32, space="PSUM")

        # pin the PE to the emitted transpose/matmul alternation: the
        # scheduler otherwise batches matmuls densely and lets transposes
        # trickle, which drops the PE HAM clock to 1.2 GHz half the time
        last_pe = [None]

        def pe(bi):
            if last_pe[0] is not None:
                tile.add_dep_helper(
                    bi.ins, last_pe[0], sync=False, reason="pe order"
                )
            last_pe[0] = bi.ins
            return bi

        # --- scoring ----------------------------------------------------
        pending_scores = []
        for g in range(NG):
            if g < PRE:
                khalf = [kt_pre[(g, 0)], kt_pre[(g, 1)]]
                vtile = vtiles[g]
            else:
                khalf = []
                for h in range(2):
                    kt = kcp.tile([128, 2, D], f32, tag="kc", name=f"kt{g}_{h}")
                    nc.sync.dma_start(kt[:], keys_g[g, h])
                    khalf.append(kt)
                vtile = vp.tile([128, 4, D], f32, tag="v", name=f"vt{g}")
                nc.sync.dma_start(vtile[:], values_g[g])
                vtiles.append(vtile)
            # c = k + v in place; h0 on DVE, h1 on GPSIMD so the two adds
            # run concurrently and neither engine gates the transposes long
            nc.vector.tensor_add(
                khalf[0][:], khalf[0][:], vtile[:, 0:2, :]
            )
            nc.gpsimd.tensor_add(
                khalf[1][:], khalf[1][:], vtile[:, 2:4, :]
            )
            # cT columns: col = 128*(2h+e) + p  ->  token 4p + 2h + e
            # two half tiles (j 0-3 / 4-7) so group g+1 can start while
            # group g's second half is still being consumed
            cTa = ctp.tile([128, NJ // 2, GT], mm_dt, tag="cT", name=f"cTa{g}")
            cTb = ctp.tile([128, NJ // 2, GT], mm_dt, tag="cT", name=f"cTb{g}")

            def cT_slice(j):
                t = cTa if j < NJ // 2 else cTb
                return t[:, j % (NJ // 2), :]
            phs = [
                ph_p.tile([128, GT], f32, tag=f"ph{hb}", name=f"ph{hb}_{g}")
                for hb in range(NHB)
            ]
            def emit_mms(j):
                for hb in range(NHB):
                    pe(
                        nc.tensor.matmul(
                            phs[hb][:],
                            w1sb[:, j, 128 * hb : 128 * (hb + 1)],
                            cT_slice(j),
                            start=(j == 0),
                            stop=(j == NJ - 1),
                        )
                    )

            # transposes run one d-block ahead of the matmuls so the PE
            # never waits on the PSUM->SBUF copy of the block it multiplies;
            # the previous group's tiny score matmuls slot into the gaps
            for j in range(NJ):
                pst = pst_p.tile([128, GT], f32, tag="pst")
                for h in range(2):
                    for e in range(2):
                        s = 2 * h + e
                        pe(
                            nc.tensor.transpose(
                                pst[:, 128 * s : 128 * (s + 1)],
                                khalf[h][:, e, 128 * j : 128 * (j + 1)],
                                ident_f32[:],
                            )
                        )
                if j % 2 == 0:
                    nc.vector.tensor_copy(cT_slice(j), pst[:])
                else:
                    nc.scalar.copy(cT_slice(j), pst[:])
                if j >= 1:
                    emit_mms(j - 1)
                if 2 <= j <= 5 and pending_scores:
                    pe(pending_scores.pop(0)())
            emit_mms(NJ - 1)
            for hb in range(NHB):
                ht = htp.tile([128, GT], f32, tag="ht")
                nc.scalar.activation(
                    ht[:],
                    phs[hb][:],
                    mybir.ActivationFunctionType.Relu,
                    bias=b1sb[:, hb : hb + 1],
                    scale=0.5,
                )
                # chunk of col = (col%128)//16; reduce pp then s
                red1 = csp.tile([128, 32], f32, tag="red1")
                nc.vector.tensor_reduce(
                    red1[:],
                    ht.rearrange("p (s pg pp) -> p s pg pp", s=4, pg=8),
                    axis=mybir.AxisListType.X,
                    op=mybir.AluOpType.add,
                )
                csum = csp.tile([128, CPG], f32, tag="csum")
                nc.vector.tensor_reduce(
                    csum[:],
                    red1.rearrange("p (s pg) -> p pg s", s=4),
                    axis=mybir.AxisListType.X,
                    op=mybir.AluOpType.add,
                )

                def make_score(g=g, hb=hb, csum=csum):
                    return nc.tensor.matmul(
                        scores_ps[0:1, CPG * g : CPG * (g + 1)],
                        w2sb[:, hb : hb + 1],
                        csum[:],
                        start=(hb == 0),
                        stop=(hb == NHB - 1),
                    )

                pending_scores.append(make_score)
        while pending_scores:
            pe(pending_scores.pop(0)())

        # --- top-32 selection (all fp32, exact) ------------------------
        scores_sb = selp.tile([1, NCH], f32, tag="sel_s")
        nc.vector.tensor_copy(scores_sb[:], scores_ps[:])

        sT_ps = psel.tile([NCH, 1], f32, space="PSUM", tag="psel")
        nc.tensor.matmul(sT_ps[:], scores_sb[:], ones_sb[:, 0:1])
        sT_sb = selp.tile([NCH, 1], f32, tag="sel_sT")
        nc.vector.tensor_copy(sT_sb[:], sT_ps[:])

        r_ps = psel.tile([NCH, NCH], f32, space="PSUM", tag="psel")
        nc.tensor.matmul(r_ps[:], ones_sb[:, :NCH], scores_sb[:])
        r_sb = selp.tile([NCH, NCH], f32, tag="sel_r")
        nc.vector.tensor_copy(r_sb[:], r_ps[:])

        g_sb = selp.tile([NCH, NCH], f32, tag="sel_g")
        nc.vector.tensor_scalar(
            g_sb[:], r_sb[:], sT_sb[:], None, op0=mybir.AluOpType.is_gt
        )
        eq_sb = selp.tile([NCH, NCH], f32, tag="sel_eq")
        nc.vector.tensor_scalar(
            eq_sb[:], r_sb[:], sT_sb[:], None, op0=mybir.AluOpType.is_equal
        )
        tie_sb = selp.tile([NCH, NCH], f32, tag="sel_tie")
        nc.vector.tensor_mul(tie_sb[:], eq_sb[:], tbm_sb)
        nc.vector.tensor_add(g_sb[:], g_sb[:], tie_sb[:])
        rank_sb = selp.tile([NCH, 1], f32, tag="sel_rank")
        nc.vector.tensor_reduce(
            rank_sb[:], g_sb[:], axis=mybir.AxisListType.X, op=mybir.AluOpType.add
        )
        keep_sb = selp.tile([NCH, 1], f32, tag="sel_keep")
        nc.vector.tensor_scalar(
            keep_sb[:], rank_sb[:], float(KEEP) - 0.5, None, op0=mybir.AluOpType.is_lt
        )

        dest_ps = psel.tile([NCH, 1], f32, space="PSUM", tag="psel")
        nc.tensor.matmul(dest_ps[:], jlt_sb, keep_sb[:])
        dest_sb = selp.tile([NCH, 1], f32, tag="sel_dest")
        nc.vector.tensor_copy(dest_sb[:], dest_ps[:])

        # M[i, o] = 1 iff chunk i goes to slot o
        sel1_sb = selp.tile([NCH, KEEP], f32, tag="sel_m")
        nc.vector.tensor_scalar(
            sel1_sb[:], iota32_sb, dest_sb[:], None, op0=mybir.AluOpType.is_equal
        )
        nc.vector.tensor_scalar(
            sel1_sb[:], sel1_sb[:], keep_sb[:], None, op0=mybir.AluOpType.mult
        )

        # --- ck: gather selected key chunks ----------------------------
        # keys viewed as [2048 rows, 2048 elems]; chunk = 32 rows; each DMA
        # moves 4 chunks: partition p fetches row 32*idx[4d + p//32] + p%32.
        keys_rows = keys.rearrange("(r q) d -> r (q d)", q=2)
        ck_rows = ck.rearrange("(r q) d -> r (q d)", q=2)

        selrep = selp.tile([NCH, NDMA * 128], f32, tag="sel_rep", bufs=1)
        for d in range(NDMA):
            nc.vector.tensor_copy(
                selrep.rearrange("i (d o u) -> i d o u", d=NDMA, u=32)[:, d],
                sel1_sb[:, GCH * d : GCH * (d + 1)].to_broadcast([NCH, GCH, 32]),
            )
        rowidx = []
        for d in range(NDMA):
            rsel_ps = psel.tile([128, 1], f32, space="PSUM", tag="psel")
            nc.tensor.matmul(rsel_ps[:], selrep[:, 128 * d : 128 * (d + 1)], iota64_sb)
            rowf = selp.tile([128, 1], f32, tag="sel_rowf")
            nc.vector.tensor_scalar(
                rowf[:],
                rsel_ps[:],
                32.0,
                iotap32_sb,
                op0=mybir.AluOpType.mult,
                op1=mybir.AluOpType.add,
            )
            ridx = selp.tile([128, 1], i32, tag=f"sel_rowi{d}")
            nc.vector.tensor_copy(ridx[:], rowf[:])
            rowidx.append(ridx)

        # --- cv scatter offsets ---------------------------------------
        # cv viewed [512 rows, 4096]; chunk = 16 rows; scatter of group g
        # writes row 16*dest[8g + p//16] + p%16 (OOB_BIG if dropped).
        comb_sb = selp.tile([NCH, 1], f32, tag="sel_comb")
        nc.vector.tensor_scalar(
            comb_sb[:],
            keep_sb[:],
            -OOB_BIG,
            OOB_BIG,
            op0=mybir.AluOpType.mult,
            op1=mybir.AluOpType.add,
        )
        tmp_sb = selp.tile([NCH, 1], f32, tag="sel_tmp")
        nc.vector.tensor_scalar(
            tmp_sb[:], dest_sb[:], 16.0, None, op0=mybir.AluOpType.mult
        )
        nc.vector.tensor_add(comb_sb[:], comb_sb[:], tmp_sb[:])

        i64rep = selp.tile([NCH, NG * 128], f32, tag="sel_rep", bufs=1)
        for g in range(NG):
            nc.vector.tensor_copy(
                i64rep.rearrange("i (g o u) -> i g o u", g=NG, u=16)[:, g],
                i64_sb[:, CPG * g : CPG * (g + 1)].to_broadcast([NCH, CPG, 16]),
            )
        voffs = []
        for g in range(NG):
            voff_ps = psel.tile([128, 1], f32, space="PSUM", tag="psel")
            nc.tensor.matmul(voff_ps[:], i64rep[:, 128 * g : 128 * (g + 1)], comb_sb[:])
            vof = selp.tile([128, 1], f32, tag="sel_vof")
            nc.vector.tensor_scalar(
                vof[:], voff_ps[:], iotap16_sb, None, op0=mybir.AluOpType.add
            )
            vofi = selp.tile([128, 1], i32, tag=f"sel_vofi{g}")
            nc.vector.tensor_copy(vofi[:], vof[:])
            voffs.append(vofi)

        # --- output DMA -------------------------------------------------
        # All ck gathers issue first: the cv scatters serialize on the cv
        # WAW chain (each waits for the previous one's completion on the
        # GpSimd queue) and would otherwise stall the gather issues too.
        cv_rows = cv.rearrange("(r q) d -> r (q d)", q=4)  # [512, 4096]
        for d in range(NDMA):
            gt = kcp.tile([128, 2048], f32, tag="kc", name=f"gt{d}")
            nc.gpsimd.indirect_dma_start(
                out=gt[:],
                out_offset=None,
                in_=keys_rows,
                in_offset=bass.IndirectOffsetOnAxis(ap=rowidx[d][:], axis=0),
            )
            nc.sync.dma_start(ck_rows[128 * d : 128 * (d + 1), :], gt[:])
        for g in range(NG):
            nc.gpsimd.indirect_dma_start(
                out=cv_rows,
                out_offset=bass.IndirectOffsetOnAxis(ap=voffs[g][:], axis=0),
                in_=vtiles[g].rearrange("p s d -> p (s d)"),
                in_offset=None,
                bounds_check=KEEP * L // 4 - 1,
                oob_is_err=False,
            )

    nc.compile()
    return nc


def _host_consts(W1, b1, W2, mode=MM_MODE):
    f32 = np.float32
    c = np.zeros((128, C_COLS), dtype=f32)
    r64 = np.arange(NCH)
    c[:NCH, C_JLT : C_JLT + NCH] = r64[:, None] < r64[None, :]
    c[:NCH, C_TBM : C_TBM + NCH] = r64[None, :] < r64[:, None]
    c[:NCH, C_I64 : C_I64 + NCH] = np.eye(NCH)
    c[:NCH, C_IOTA32 : C_IOTA32 + KEEP] = np.arange(KEEP)[None, :]
    c[:NCH, C_IOTA64] = r64
    c[:, C_IOTAP32] = np.arange(128) % 32
    c[:, C_IOTAP16] = np.arange(128) % 16
    c[:, C_B1 : C_B1 + NHB] = np.asarray(b1, f32).reshape(NHB, 128).T
    c[:, C_W2 : C_W2 + NHB] = np.asarray(W2, f32)[:, 0].reshape(NHB, 128).T
    c[0, C_ONES : C_ONES + 128] = 1.0
    return {
        "w1": np.ascontiguousarray(W1, dtype=f32),
        "ident": np.eye(128, dtype=f32),
        "consts": c,
    }


def get_nc(mode=MM_MODE):
    key = ("nc", mode)
    if key not in _CACHE:
        _CACHE[key] = _build(mode)
    return _CACHE[key]


def kernel(keys, values, W1, b1, W2, b2):
    from concourse.bass_utils import run_bass_kernel_spmd

    nc = get_nc()
    keys = np.asarray(keys)
    values = np.asarray(values)
    consts = _host_consts(np.asarray(W1), np.asarray(b1), np.asarray(W2))
    in_maps = [dict(keys=keys[i], values=values[i], **consts) for i in range(B)]
    res = run_bass_kernel_spmd(nc, in_maps, list(range(B)))
    ck = np.stack([res.results[i]["ck"] for i in range(B)])
    cv = np.stack([res.results[i]["cv"] for i in range(B)])
    return ck, cv


# revision 45
# speedup vs baseline: 1.1652x; 1.0531x over previous
"""ChunkKVCompressor Trainium2 kernel.

Data-parallel over batch: core i handles batch element i (B=8 across 8 cores).
Per core:
  1. keys/values stream in 8 groups of 512 tokens. values tiles stay RESIDENT
     in SBUF (16 MB) in a quad layout (partition p holds tokens 4p..4p+3 of
     its group) so compressed values scatter straight from SBUF.
     keys load in two matching pair-layout half tiles per group.
  2. c = k + v in-place into the k tiles (DVE), rounded to float32r.
  3. Per d-block: PE transposes (4 blocks) then immediately 4 h-block
     float32r matmuls (N=512, LDWEIGHTS hidden) accumulating into 4 PSUM
     banks -- fine interleave keeps the PE HAM clock warm.
  4. relu(0.5*x + b1) on ACT; chunk sums via two DVE segment reduces
     (token order inside cT is 4*(col%128) + col//128); scores += W2.T@sums
     into one persistent PSUM row [1, 64] (mean/b2 dropped: order-preserving).
  5. On-device top-32-of-64 by rank (comparison matrix + tiny matmuls).
  6. ck: 8x 1MB indirect gathers (one row index per partition, source viewed
     as [2048 rows x 2048 elems]) staged through SBUF + contiguous writes.
     cv: 8x 2MB indirect scatters straight from the resident value tiles
     (cv viewed as [512 rows x 4096 elems]); dropped chunks get row 8192
     and are skipped via bounds_check.
"""

import sys

if "/opt/trn_rl_repo" not in sys.path:
    sys.path.insert(0, "/opt/trn_rl_repo")

import numpy as np
from contextlib import ExitStack

B, T, D, H = 8, 4096, 1024, 512
L = 64  # chunk length (tokens)
NCH = T // L  # 64 chunks
KEEP = 32  # chunks kept per batch
NG = 8  # token groups
GT = T // NG  # 512 tokens per group
NJ = D // 128  # 8 d-blocks
NHB = H // 128  # 4 h-blocks
CPG = NCH // NG  # 8 chunks per group
GCH = 4  # chunks per k-gather DMA
NDMA = KEEP // GCH  # 8 k-gather DMAs
MM_MODE = "f32r"  # "f32r" | "f32"
OOB_BIG = 8192.0  # scatter row for dropped chunks (valid rows 0..511)

# packed fp32 const tensor column layout [128, 384]
C_JLT = 0
C_TBM = 64
C_I64 = 128
C_IOTA32 = 192
C_IOTA64 = 224
C_IOTAP32 = 225
C_IOTAP16 = 226
C_B1 = 229
C_W2 = 233
C_ONES = 240  # [0:1, 240:368]
C_COLS = 384

_CACHE = {}


def _build(mode=MM_MODE):
    import concourse.bass as bass
    import concourse.tile as tile
    from concourse import bacc, mybir

    f32 = mybir.dt.float32
    i32 = mybir.dt.int32
    mm_dt = {"f32r": mybir.dt.float32r, "f32": f32}[mode]

    nc = bacc.Bacc("TRN2", target_bir_lowering=False, debug=False, num_devices=B)

    keys = nc.dram_tensor("keys", [T, D], f32, kind="ExternalInput").ap()
    values = nc.dram_tensor("values", [T, D], f32, kind="ExternalInput").ap()
    w1 = nc.dram_tensor("w1", [D, H], mm_dt, kind="ExternalInput").ap()
    ident_d = nc.dram_tensor("ident", [128, 128], f32, kind="ExternalInput").ap()
    consts_d = nc.dram_tensor("consts", [128, C_COLS], f32, kind="ExternalInput").ap()
    ck = nc.dram_tensor("ck", [KEEP * L, D], f32, kind="ExternalOutput").ap()
    cv = nc.dram_tensor("cv", [KEEP * L, D], f32, kind="ExternalOutput").ap()

    def cast(ap):
        return ap.bitcast(mm_dt) if mode == "f32r" else ap

    with tile.TileContext(nc) as tc, ExitStack() as ctx:
        wp = ctx.enter_context(tc.tile_pool(name="wp", bufs=1))
        kcp = ctx.enter_context(tc.tile_pool(name="kcp", bufs=4))
        vp = ctx.enter_context(tc.tile_pool(name="vp", bufs=NG))
        ctp = ctx.enter_context(tc.tile_pool(name="ctp", bufs=2))
        htp = ctx.enter_context(tc.tile_pool(name="htp", bufs=1))
        csp = ctx.enter_context(tc.tile_pool(name="csp", bufs=8))
        selp = ctx.enter_context(tc.tile_pool(name="selp", bufs=1))
        pst_p = ctx.enter_context(tc.tile_pool(name="pst", bufs=2, space="PSUM"))
        ph_p = ctx.enter_context(tc.tile_pool(name="ph", bufs=1, space="PSUM"))
        sc_p = ctx.enter_context(tc.tile_pool(name="sc", bufs=1, space="PSUM"))
        psel = ctx.enter_context(tc.tile_pool(name="psel", bufs=1, space="PSUM"))

        # --- constants / weights to SBUF (one packed DMA + ident + w1) --
        consts = wp.tile([128, C_COLS], f32)
        nc.sync.dma_start(consts[:], consts_d[:])
        ident_sb = wp.tile([128, 128], mm_dt)
        nc.sync.dma_start(ident_sb[:], ident_d[:].bitcast(mm_dt))

        jlt_sb = consts[:NCH, C_JLT : C_JLT + NCH]
        tbm_sb = consts[:NCH, C_TBM : C_TBM + NCH]
        i64_sb = consts[:NCH, C_I64 : C_I64 + NCH]
        iota32_sb = consts[:NCH, C_IOTA32 : C_IOTA32 + KEEP]
        iota64_sb = consts[:NCH, C_IOTA64 : C_IOTA64 + 1]
        iotap32_sb = consts[:, C_IOTAP32 : C_IOTAP32 + 1]
        iotap16_sb = consts[:, C_IOTAP16 : C_IOTAP16 + 1]
        b1sb = consts[:, C_B1 : C_B1 + NHB]
        w2sb = consts[:, C_W2 : C_W2 + NHB]
        ones_sb = consts[0:1, C_ONES : C_ONES + 128]

        # token tau = 512g + 4p + 2h + e  (k half tiles, h in {0,1})
        keys_g = keys.rearrange("(g p h e) d -> g h p e d", p=128, h=2, e=2)
        # token tau = 512g + 4p + s      (v quad tiles)
        values_g = values.rearrange("(g p s) d -> g p s d", p=128, s=4)

        # preload first group's k/v before w1 so compute starts immediately
        kt_pre = {}
        vtiles = []
        PRE = 2
        for g in range(PRE):
            for h in range(2):
                kt = kcp.tile([128, 2, D], mm_dt, tag="kc", name=f"kt{g}_{h}")
                nc.sync.dma_start(kt[:], cast(keys_g[g, h]))
                kt_pre[(g, h)] = kt
            vt = vp.tile([128, 4, D], f32, tag="v", name=f"vt{g}")
            nc.sync.dma_start(vt[:], values_g[g])
            vtiles.append(vt)

        w1sb = wp.tile([128, NJ, H], mm_dt)  # [p, j, hh]; d = j*128 + p
        nc.sync.dma_start(w1sb[:], w1.rearrange("(j p) hh -> p j hh", p=128))

        scores_ps = sc_p.tile([1, NCH], f32, space="PSUM")

        # pin the PE to the emitted transpose/matmul alternation: the
        # scheduler otherwise batches matmuls densely and lets transposes
        # trickle, which drops the PE HAM clock to 1.2 GHz half the time
        last_pe = [None]

        def pe(bi):
            return bi

        # --- scoring ----------------------------------------------------
        pending_scores = []
        for g in range(NG):
            if g < PRE:
                khalf = [kt_pre[(g, 0)], kt_pre[(g, 1)]]
                vtile = vtiles[g]
            else:
                khalf = []
                for h in range(2):
                    kt = kcp.tile([128, 2, D], mm_dt, tag="kc", name=f"kt{g}_{h}")
                    nc.sync.dma_start(kt[:], cast(keys_g[g, h]))
                    khalf.append(kt)
                vtile = vp.tile([128, 4, D], f32, tag="v", name=f"vt{g}")
                nc.sync.dma_start(vtile[:], values_g[g])
                vtiles.append(vtile)
            # c = k + v in place; h0 on DVE, h1 on GPSIMD so the two adds
            # run concurrently and neither engine gates the transposes long
            nc.vector.tensor_add(
                khalf[0][:], khalf[0][:], vtile[:, 0:2, :]
            )
            nc.gpsimd.tensor_add(
                khalf[1][:], khalf[1][:], vtile[:, 2:4, :]
            )
            # cT columns: col = 128*(2h+e) + p  ->  token 4p + 2h + e
            # two half tiles (j 0-3 / 4-7) so group g+1 can start while
            # group g's second half is still being consumed
            cTa = ctp.tile([128, NJ // 2, GT], mm_dt, tag="cT", name=f"cTa{g}")
            cTb = ctp.tile([128, NJ // 2, GT], mm_dt, tag="cT", name=f"cTb{g}")

            def cT_slice(j):
                t = cTa if j < NJ // 2 else cTb
                return t[:, j % (NJ // 2), :]
            phs = [
                ph_p.tile([128, GT], f32, tag=f"ph{hb}", name=f"ph{hb}_{g}")
                for hb in range(NHB)
            ]
            def emit_mms(j):
                for hb in range(NHB):
                    pe(
                        nc.tensor.matmul(
                            phs[hb][:],
                            w1sb[:, j, 128 * hb : 128 * (hb + 1)],
                            cT_slice(j),
                            start=(j == 0),
                            stop=(j == NJ - 1),
                        )
                    )

            # transposes run one d-block ahead of the matmuls so the PE
            # never waits on the PSUM->SBUF copy of the block it multiplies;
            # the previous group's tiny score matmuls slot into the gaps
            for j in range(NJ):
                pst = pst_p.tile([128, GT], mm_dt, tag="pst")
                for h in range(2):
                    for e in range(2):
                        s = 2 * h + e
                        pe(
                            nc.tensor.transpose(
                                pst[:, 128 * s : 128 * (s + 1)],
                                khalf[h][:, e, 128 * j : 128 * (j + 1)],
                                ident_sb[:],
                            )
                        )
                if j % 2 == 0:
                    nc.vector.tensor_copy(cT_slice(j), pst[:])
                else:
                    nc.scalar.copy(cT_slice(j), pst[:])
                if j >= 1:
                    emit_mms(j - 1)
                if 2 <= j <= 5 and pending_scores:
                    pe(pending_scores.pop(0)())
            emit_mms(NJ - 1)
            for hb in range(NHB):
                ht = htp.tile([128, GT], f32, tag="ht")
                nc.scalar.activation(
                    ht[:],
                    phs[hb][:],
                    mybir.ActivationFunctionType.Relu,
                    bias=b1sb[:, hb : hb + 1],
                    scale=0.5,
                )
                # chunk of col = (col%128)//16; reduce pp then s
                red1 = csp.tile([128, 32], f32, tag="red1")
                nc.vector.tensor_reduce(
                    red1[:],
                    ht.rearrange("p (s pg pp) -> p s pg pp", s=4, pg=8),
                    axis=mybir.AxisListType.X,
                    op=mybir.AluOpType.add,
                )
                csum = csp.tile([128, CPG], f32, tag="csum")
                nc.vector.tensor_reduce(
                    csum[:],
                    red1.rearrange("p (s pg) -> p pg s", s=4),
                    axis=mybir.AxisListType.X,
                    op=mybir.AluOpType.add,
                )

                def make_score(g=g, hb=hb, csum=csum):
                    return nc.tensor.matmul(
                        scores_ps[0:1, CPG * g : CPG * (g + 1)],
                        w2sb[:, hb : hb + 1],
                        csum[:],
                        start=(hb == 0),
                        stop=(hb == NHB - 1),
                    )

                pending_scores.append(make_score)
        while pending_scores:
            pe(pending_scores.pop(0)())

        # --- top-32 selection (all fp32, exact) ------------------------
        scores_sb = selp.tile([1, NCH], f32, tag="sel_s")
        nc.vector.tensor_copy(scores_sb[:], scores_ps[:])

        sT_ps = psel.tile([NCH, 1], f32, space="PSUM", tag="psel")
        nc.tensor.matmul(sT_ps[:], scores_sb[:], ones_sb[:, 0:1])
        sT_sb = selp.tile([NCH, 1], f32, tag="sel_sT")
        nc.vector.tensor_copy(sT_sb[:], sT_ps[:])

        r_ps = psel.tile([NCH, NCH], f32, space="PSUM", tag="psel")
        nc.tensor.matmul(r_ps[:], ones_sb[:, :NCH], scores_sb[:])
        r_sb = selp.tile([NCH, NCH], f32, tag="sel_r")
        nc.vector.tensor_copy(r_sb[:], r_ps[:])

        g_sb = selp.tile([NCH, NCH], f32, tag="sel_g")
        nc.vector.tensor_scalar(
            g_sb[:], r_sb[:], sT_sb[:], None, op0=mybir.AluOpType.is_gt
        )
        eq_sb = selp.tile([NCH, NCH], f32, tag="sel_eq")
        nc.vector.tensor_scalar(
            eq_sb[:], r_sb[:], sT_sb[:], None, op0=mybir.AluOpType.is_equal
        )
        tie_sb = selp.tile([NCH, NCH], f32, tag="sel_tie")
        nc.vector.tensor_mul(tie_sb[:], eq_sb[:], tbm_sb)
        nc.vector.tensor_add(g_sb[:], g_sb[:], tie_sb[:])
        rank_sb = selp.tile([NCH, 1], f32, tag="sel_rank")
        nc.vector.tensor_reduce(
            rank_sb[:], g_sb[:], axis=mybir.AxisListType.X, op=mybir.AluOpType.add
        )
        keep_sb = selp.tile([NCH, 1], f32, tag="sel_keep")
        nc.vector.tensor_scalar(
            keep_sb[:], rank_sb[:], float(KEEP) - 0.5, None, op0=mybir.AluOpType.is_lt
        )

        dest_ps = psel.tile([NCH, 1], f32, space="PSUM", tag="psel")
        nc.tensor.matmul(dest_ps[:], jlt_sb, keep_sb[:])
        dest_sb = selp.tile([NCH, 1], f32, tag="sel_dest")
        nc.vector.tensor_copy(dest_sb[:], dest_ps[:])

        # M[i, o] = 1 iff chunk i goes to slot o
        sel1_sb = selp.tile([NCH, KEEP], f32, tag="sel_m")
        nc.vector.tensor_scalar(
            sel1_sb[:], iota32_sb, dest_sb[:], None, op0=mybir.AluOpType.is_equal
        )
        nc.vector.tensor_scalar(
            sel1_sb[:], sel1_sb[:], keep_sb[:], None, op0=mybir.AluOpType.mult
        )

        # --- ck: gather selected key chunks ----------------------------
        # keys viewed as [2048 rows, 2048 elems]; chunk = 32 rows; each DMA
        # moves 4 chunks: partition p fetches row 32*idx[4d + p//32] + p%32.
        keys_rows = keys.rearrange("(r q) d -> r (q d)", q=2)
        ck_rows = ck.rearrange("(r q) d -> r (q d)", q=2)

        selrep = selp.tile([NCH, NDMA * 128], f32, tag="sel_rep", bufs=1)
        for d in range(NDMA):
            nc.vector.tensor_copy(
                selrep.rearrange("i (d o u) -> i d o u", d=NDMA, u=32)[:, d],
                sel1_sb[:, GCH * d : GCH * (d + 1)].to_broadcast([NCH, GCH, 32]),
            )
        rowidx = []
        for d in range(NDMA):
            rsel_ps = psel.tile([128, 1], f32, space="PSUM", tag="psel")
            nc.tensor.matmul(rsel_ps[:], selrep[:, 128 * d : 128 * (d + 1)], iota64_sb)
            rowf = selp.tile([128, 1], f32, tag="sel_rowf")
            nc.vector.tensor_scalar(
                rowf[:],
                rsel_ps[:],
                32.0,
                iotap32_sb,
                op0=mybir.AluOpType.mult,
                op1=mybir.AluOpType.add,
            )
            ridx = selp.tile([128, 1], i32, tag=f"sel_rowi{d}")
            nc.vector.tensor_copy(ridx[:], rowf[:])
            rowidx.append(ridx)

        # --- cv scatter offsets ---------------------------------------
        # cv viewed [512 rows, 4096]; chunk = 16 rows; scatter of group g
        # writes row 16*dest[8g + p//16] + p%16 (OOB_BIG if dropped).
        comb_sb = selp.tile([NCH, 1], f32, tag="sel_comb")
        nc.vector.tensor_scalar(
            comb_sb[:],
            keep_sb[:],
            -OOB_BIG,
            OOB_BIG,
            op0=mybir.AluOpType.mult,
            op1=mybir.AluOpType.add,
        )
        tmp_sb = selp.tile([NCH, 1], f32, tag="sel_tmp")
        nc.vector.tensor_scalar(
            tmp_sb[:], dest_sb[:], 16.0, None, op0=mybir.AluOpType.mult
        )
        nc.vector.tensor_add(comb_sb[:], comb_sb[:], tmp_sb[:])

        i64rep = selp.tile([NCH, NG * 128], f32, tag="sel_rep", bufs=1)
        for g in range(NG):
            nc.vector.tensor_copy(
                i64rep.rearrange("i (g o u) -> i g o u", g=NG, u=16)[:, g],
                i64_sb[:, CPG * g : CPG * (g + 1)].to_broadcast([NCH, CPG, 16]),
            )
        voffs = []
        for g in range(NG):
            voff_ps = psel.tile([128, 1], f32, space="PSUM", tag="psel")
            nc.tensor.matmul(voff_ps[:], i64rep[:, 128 * g : 128 * (g + 1)], comb_sb[:])
            vof = selp.tile([128, 1], f32, tag="sel_vof")
            nc.vector.tensor_scalar(
                vof[:], voff_ps[:], iotap16_sb, None, op0=mybir.AluOpType.add
            )
            vofi = selp.tile([128, 1], i32, tag=f"sel_vofi{g}")
            nc.vector.tensor_copy(vofi[:], vof[:])
            voffs.append(vofi)

        # --- output DMA -------------------------------------------------
        # All ck gathers issue first: the cv scatters serialize on the cv
        # WAW chain (each waits for the previous one's completion on the
        # GpSimd queue) and would otherwise stall the gather issues too.
        cv_rows = cv.rearrange("(r q) d -> r (q d)", q=4)  # [512, 4096]
        for d in range(NDMA):
            gt = kcp.tile([128, 2048], f32, tag="kc", name=f"gt{d}")
            nc.gpsimd.indirect_dma_start(
                out=gt[:],
                out_offset=None,
                in_=keys_rows,
                in_offset=bass.IndirectOffsetOnAxis(ap=rowidx[d][:], axis=0),
            )
            nc.sync.dma_start(ck_rows[128 * d : 128 * (d + 1), :], gt[:])
        for g in range(NG):
            nc.gpsimd.indirect_dma_start(
                out=cv_rows,
                out_offset=bass.IndirectOffsetOnAxis(ap=voffs[g][:], axis=0),
                in_=vtiles[g].rearrange("p s d -> p (s d)"),
                in_offset=None,
                bounds_check=KEEP * L // 4 - 1,
                oob_is_err=False,
            )

    nc.compile()
    return nc


def _host_consts(W1, b1, W2, mode=MM_MODE):
    f32 = np.float32
    c = np.zeros((128, C_COLS), dtype=f32)
    r64 = np.arange(NCH)
    c[:NCH, C_JLT : C_JLT + NCH] = r64[:, None] < r64[None, :]
    c[:NCH, C_TBM : C_TBM + NCH] = r64[None, :] < r64[:, None]
    c[:NCH, C_I64 : C_I64 + NCH] = np.eye(NCH)
    c[:NCH, C_IOTA32 : C_IOTA32 + KEEP] = np.arange(KEEP)[None, :]
    c[:NCH, C_IOTA64] = r64
    c[:, C_IOTAP32] = np.arange(128) % 32
    c[:, C_IOTAP16] = np.arange(128) % 16
    c[:, C_B1 : C_B1 + NHB] = np.asarray(b1, f32).reshape(NHB, 128).T
    c[:, C_W2 : C_W2 + NHB] = np.asarray(W2, f32)[:, 0].reshape(NHB, 128).T
    c[0, C_ONES : C_ONES + 128] = 1.0
    return {
        "w1": np.ascontiguousarray(W1, dtype=f32),
        "ident": np.eye(128, dtype=f32),
        "consts": c,
    }


def get_nc(mode=MM_MODE):
    key = ("nc", mode)
    if key not in _CACHE:
        _CACHE[key] = _build(mode)
    return _CACHE[key]


def kernel(keys, values, W1, b1, W2, b2):
    from concourse.bass_utils import run_bass_kernel_spmd

    nc = get_nc()
    keys = np.asarray(keys)
    values = np.asarray(values)
    consts = _host_consts(np.asarray(W1), np.asarray(b1), np.asarray(W2))
    in_maps = [dict(keys=keys[i], values=values[i], **consts) for i in range(B)]
    res = run_bass_kernel_spmd(nc, in_maps, list(range(B)))
    ck = np.stack([res.results[i]["ck"] for i in range(B)])
    cv = np.stack([res.results[i]["cv"] for i in range(B)])
    return ck, cv


# revision 46
# speedup vs baseline: 1.2441x; 1.0677x over previous
"""ChunkKVCompressor Trainium2 kernel.

Data-parallel over batch: core i handles batch element i (B=8 across 8 cores).
Per core:
  1. keys/values stream in 8 groups of 512 tokens. values tiles stay RESIDENT
     in SBUF (16 MB) in a quad layout (partition p holds tokens 4p..4p+3 of
     its group) so compressed values scatter straight from SBUF.
     keys load in two matching pair-layout half tiles per group.
  2. c = k + v in-place into the k tiles (DVE), rounded to float32r.
  3. Per d-block: PE transposes (4 blocks) then immediately 4 h-block
     float32r matmuls (N=512, LDWEIGHTS hidden) accumulating into 4 PSUM
     banks -- fine interleave keeps the PE HAM clock warm.
  4. relu(0.5*x + b1) on ACT; chunk sums via two DVE segment reduces
     (token order inside cT is 4*(col%128) + col//128); scores += W2.T@sums
     into one persistent PSUM row [1, 64] (mean/b2 dropped: order-preserving).
  5. On-device top-32-of-64 by rank (comparison matrix + tiny matmuls).
  6. ck: 8x 1MB indirect gathers (one row index per partition, source viewed
     as [2048 rows x 2048 elems]) staged through SBUF + contiguous writes.
     cv: 8x 2MB indirect scatters straight from the resident value tiles
     (cv viewed as [512 rows x 4096 elems]); dropped chunks get row 8192
     and are skipped via bounds_check.
"""

import sys

if "/opt/trn_rl_repo" not in sys.path:
    sys.path.insert(0, "/opt/trn_rl_repo")

import numpy as np
from contextlib import ExitStack

B, T, D, H = 8, 4096, 1024, 512
L = 64  # chunk length (tokens)
NCH = T // L  # 64 chunks
KEEP = 32  # chunks kept per batch
NG = 8  # token groups
GT = T // NG  # 512 tokens per group
NJ = D // 128  # 8 d-blocks
NHB = H // 128  # 4 h-blocks
CPG = NCH // NG  # 8 chunks per group
GCH = 4  # chunks per k-gather DMA
NDMA = KEEP // GCH  # 8 k-gather DMAs
MM_MODE = "f32r"  # "f32r" | "f32"
OOB_BIG = 8192.0  # scatter row for dropped chunks (valid rows 0..511)

# packed fp32 const tensor column layout [128, 384]
C_JLT = 0
C_TBM = 64
C_I64 = 128
C_IOTA32 = 192
C_IOTA64 = 224
C_IOTAP32 = 225
C_IOTAP16 = 226
C_B1 = 229
C_W2 = 233
C_ONES = 240  # [0:1, 240:368]
C_COLS = 384

_CACHE = {}


def _build(mode=MM_MODE):
    import concourse.bass as bass
    import concourse.tile as tile
    from concourse import bacc, mybir

    f32 = mybir.dt.float32
    i32 = mybir.dt.int32
    mm_dt = {"f32r": mybir.dt.float32r, "f32": f32}[mode]

    nc = bacc.Bacc("TRN2", target_bir_lowering=False, debug=False, num_devices=B)

    keys = nc.dram_tensor("keys", [T, D], f32, kind="ExternalInput").ap()
    values = nc.dram_tensor("values", [T, D], f32, kind="ExternalInput").ap()
    w1 = nc.dram_tensor("w1", [D, H], mm_dt, kind="ExternalInput").ap()
    ident_d = nc.dram_tensor("ident", [128, 128], f32, kind="ExternalInput").ap()
    consts_d = nc.dram_tensor("consts", [128, C_COLS], f32, kind="ExternalInput").ap()
    ck = nc.dram_tensor("ck", [KEEP * L, D], f32, kind="ExternalOutput").ap()
    cv = nc.dram_tensor("cv", [KEEP * L, D], f32, kind="ExternalOutput").ap()

    def cast(ap):
        return ap.bitcast(mm_dt) if mode == "f32r" else ap

    with tile.TileContext(nc) as tc, ExitStack() as ctx:
        wp = ctx.enter_context(tc.tile_pool(name="wp", bufs=1))
        kcp = ctx.enter_context(tc.tile_pool(name="kcp", bufs=4))
        vp = ctx.enter_context(tc.tile_pool(name="vp", bufs=NG))
        ctp = ctx.enter_context(tc.tile_pool(name="ctp", bufs=2))
        htp = ctx.enter_context(tc.tile_pool(name="htp", bufs=1))
        csp = ctx.enter_context(tc.tile_pool(name="csp", bufs=8))
        selp = ctx.enter_context(tc.tile_pool(name="selp", bufs=1))
        pst_p = ctx.enter_context(tc.tile_pool(name="pst", bufs=2, space="PSUM"))
        ph_p = ctx.enter_context(tc.tile_pool(name="ph", bufs=1, space="PSUM"))
        sc_p = ctx.enter_context(tc.tile_pool(name="sc", bufs=1, space="PSUM"))
        psel = ctx.enter_context(tc.tile_pool(name="psel", bufs=1, space="PSUM"))

        # --- constants / weights to SBUF (one packed DMA + ident + w1) --
        consts = wp.tile([128, C_COLS], f32)
        nc.sync.dma_start(consts[:], consts_d[:])
        ident_sb = wp.tile([128, 128], mm_dt)
        nc.sync.dma_start(ident_sb[:], ident_d[:].bitcast(mm_dt))

        jlt_sb = consts[:NCH, C_JLT : C_JLT + NCH]
        tbm_sb = consts[:NCH, C_TBM : C_TBM + NCH]
        i64_sb = consts[:NCH, C_I64 : C_I64 + NCH]
        iota32_sb = consts[:NCH, C_IOTA32 : C_IOTA32 + KEEP]
        iota64_sb = consts[:NCH, C_IOTA64 : C_IOTA64 + 1]
        iotap32_sb = consts[:, C_IOTAP32 : C_IOTAP32 + 1]
        iotap16_sb = consts[:, C_IOTAP16 : C_IOTAP16 + 1]
        b1sb = consts[:, C_B1 : C_B1 + NHB]
        w2sb = consts[:, C_W2 : C_W2 + NHB]
        ones_sb = consts[0:1, C_ONES : C_ONES + 128]

        # token tau = 512g + 4p + 2h + e  (k half tiles, h in {0,1})
        keys_g = keys.rearrange("(g p h e) d -> g h p e d", p=128, h=2, e=2)
        # token tau = 512g + 4p + s      (v quad tiles)
        values_g = values.rearrange("(g p s) d -> g p s d", p=128, s=4)

        # preload first group's k/v before w1 so compute starts immediately
        kt_pre = {}
        vtiles = []
        PRE = 2
        for g in range(PRE):
            for h in range(2):
                kt = kcp.tile([128, 2, D], mm_dt, tag="kc", name=f"kt{g}_{h}")
                nc.sync.dma_start(kt[:], cast(keys_g[g, h]))
                kt_pre[(g, h)] = kt
            vt = vp.tile([128, 4, D], f32, tag="v", name=f"vt{g}")
            nc.sync.dma_start(vt[:], values_g[g])
            vtiles.append(vt)

        w1sb = wp.tile([128, NJ, H], mm_dt)  # [p, j, hh]; d = j*128 + p
        nc.sync.dma_start(w1sb[:], w1.rearrange("(j p) hh -> p j hh", p=128))

        scores_ps = sc_p.tile([1, NCH], f32, space="PSUM")

        # pin the PE to the emitted transpose/matmul alternation: the
        # scheduler otherwise batches matmuls densely and lets transposes
        # trickle, which drops the PE HAM clock to 1.2 GHz half the time
        last_pe = [None]

        def pe(bi):
            return bi

        # --- scoring ----------------------------------------------------
        pending_scores = []
        for g in range(NG):
            if g < PRE:
                khalf = [kt_pre[(g, 0)], kt_pre[(g, 1)]]
                vtile = vtiles[g]
            else:
                khalf = []
                for h in range(2):
                    kt = kcp.tile([128, 2, D], mm_dt, tag="kc", name=f"kt{g}_{h}")
                    nc.sync.dma_start(kt[:], cast(keys_g[g, h]))
                    khalf.append(kt)
                vtile = vp.tile([128, 4, D], f32, tag="v", name=f"vt{g}")
                nc.sync.dma_start(vtile[:], values_g[g])
                vtiles.append(vtile)
            # c = k + v in place; h0 on DVE, h1 on GPSIMD so the two adds
            # run concurrently and neither engine gates the transposes long
            nc.vector.tensor_add(
                khalf[0][:], khalf[0][:], vtile[:, 0:2, :]
            )
            nc.gpsimd.tensor_add(
                khalf[1][:], khalf[1][:], vtile[:, 2:4, :]
            )
            # cT columns: col = 128*(2h+e) + p  ->  token 4p + 2h + e
            # two half tiles (j 0-3 / 4-7) so group g+1 can start while
            # group g's second half is still being consumed
            cTa = ctp.tile([128, NJ // 2, GT], mm_dt, tag="cT", name=f"cTa{g}")
            cTb = ctp.tile([128, NJ // 2, GT], mm_dt, tag="cT", name=f"cTb{g}")

            def cT_slice(j):
                t = cTa if j < NJ // 2 else cTb
                return t[:, j % (NJ // 2), :]
            phs = [
                ph_p.tile([128, GT], f32, tag=f"ph{hb}", name=f"ph{hb}_{g}")
                for hb in range(NHB)
            ]
            def emit_mms_hb(hb):
                for j in range(NJ):
                    pe(
                        nc.tensor.matmul(
                            phs[hb][:],
                            w1sb[:, j, 128 * hb : 128 * (hb + 1)],
                            cT_slice(j),
                            start=(j == 0),
                            stop=(j == NJ - 1),
                        )
                    )

            # transposes run one d-block ahead of the matmuls so the PE
            # never waits on the PSUM->SBUF copy of the block it multiplies;
            # the previous group's tiny score matmuls slot into the gaps
            for j in range(NJ):
                pst = pst_p.tile([128, GT], mm_dt, tag="pst")
                for h in range(2):
                    for e in range(2):
                        s = 2 * h + e
                        pe(
                            nc.tensor.transpose(
                                pst[:, 128 * s : 128 * (s + 1)],
                                khalf[h][:, e, 128 * j : 128 * (j + 1)],
                                ident_sb[:],
                            )
                        )
                if j % 2 == 0:
                    nc.vector.tensor_copy(cT_slice(j), pst[:])
                else:
                    nc.scalar.copy(cT_slice(j), pst[:])
            for hb in range(NHB):
                emit_mms_hb(hb)
                if pending_scores:
                    pe(pending_scores.pop(0)())
            for hb in range(NHB):
                ht = htp.tile([128, GT], f32, tag="ht")
                nc.scalar.activation(
                    ht[:],
                    phs[hb][:],
                    mybir.ActivationFunctionType.Relu,
                    bias=b1sb[:, hb : hb + 1],
                    scale=0.5,
                )
                # chunk of col = (col%128)//16; reduce pp then s
                red1 = csp.tile([128, 32], f32, tag="red1")
                nc.vector.tensor_reduce(
                    red1[:],
                    ht.rearrange("p (s pg pp) -> p s pg pp", s=4, pg=8),
                    axis=mybir.AxisListType.X,
                    op=mybir.AluOpType.add,
                )
                csum = csp.tile([128, CPG], f32, tag="csum")
                nc.vector.tensor_reduce(
                    csum[:],
                    red1.rearrange("p (s pg) -> p pg s", s=4),
                    axis=mybir.AxisListType.X,
                    op=mybir.AluOpType.add,
                )

                def make_score(g=g, hb=hb, csum=csum):
                    return nc.tensor.matmul(
                        scores_ps[0:1, CPG * g : CPG * (g + 1)],
                        w2sb[:, hb : hb + 1],
                        csum[:],
                        start=(hb == 0),
                        stop=(hb == NHB - 1),
                    )

                pending_scores.append(make_score)
        while pending_scores:
            pe(pending_scores.pop(0)())

        # --- top-32 selection (all fp32, exact) ------------------------
        scores_sb = selp.tile([1, NCH], f32, tag="sel_s")
        nc.vector.tensor_copy(scores_sb[:], scores_ps[:])

        sT_ps = psel.tile([NCH, 1], f32, space="PSUM", tag="psel")
        nc.tensor.matmul(sT_ps[:], scores_sb[:], ones_sb[:, 0:1])
        sT_sb = selp.tile([NCH, 1], f32, tag="sel_sT")
        nc.vector.tensor_copy(sT_sb[:], sT_ps[:])

        r_ps = psel.tile([NCH, NCH], f32, space="PSUM", tag="psel")
        nc.tensor.matmul(r_ps[:], ones_sb[:, :NCH], scores_sb[:])
        r_sb = selp.tile([NCH, NCH], f32, tag="sel_r")
        nc.vector.tensor_copy(r_sb[:], r_ps[:])

        g_sb = selp.tile([NCH, NCH], f32, tag="sel_g")
        nc.vector.tensor_scalar(
            g_sb[:], r_sb[:], sT_sb[:], None, op0=mybir.AluOpType.is_gt
        )
        eq_sb = selp.tile([NCH, NCH], f32, tag="sel_eq")
        nc.vector.tensor_scalar(
            eq_sb[:], r_sb[:], sT_sb[:], None, op0=mybir.AluOpType.is_equal
        )
        tie_sb = selp.tile([NCH, NCH], f32, tag="sel_tie")
        nc.vector.tensor_mul(tie_sb[:], eq_sb[:], tbm_sb)
        nc.vector.tensor_add(g_sb[:], g_sb[:], tie_sb[:])
        rank_sb = selp.tile([NCH, 1], f32, tag="sel_rank")
        nc.vector.tensor_reduce(
            rank_sb[:], g_sb[:], axis=mybir.AxisListType.X, op=mybir.AluOpType.add
        )
        keep_sb = selp.tile([NCH, 1], f32, tag="sel_keep")
        nc.vector.tensor_scalar(
            keep_sb[:], rank_sb[:], float(KEEP) - 0.5, None, op0=mybir.AluOpType.is_lt
        )

        dest_ps = psel.tile([NCH, 1], f32, space="PSUM", tag="psel")
        nc.tensor.matmul(dest_ps[:], jlt_sb, keep_sb[:])
        dest_sb = selp.tile([NCH, 1], f32, tag="sel_dest")
        nc.vector.tensor_copy(dest_sb[:], dest_ps[:])

        # M[i, o] = 1 iff chunk i goes to slot o
        sel1_sb = selp.tile([NCH, KEEP], f32, tag="sel_m")
        nc.vector.tensor_scalar(
            sel1_sb[:], iota32_sb, dest_sb[:], None, op0=mybir.AluOpType.is_equal
        )
        nc.vector.tensor_scalar(
            sel1_sb[:], sel1_sb[:], keep_sb[:], None, op0=mybir.AluOpType.mult
        )

        # --- ck: gather selected key chunks ----------------------------
        # keys viewed as [2048 rows, 2048 elems]; chunk = 32 rows; each DMA
        # moves 4 chunks: partition p fetches row 32*idx[4d + p//32] + p%32.
        keys_rows = keys.rearrange("(r q) d -> r (q d)", q=2)
        ck_rows = ck.rearrange("(r q) d -> r (q d)", q=2)

        selrep = selp.tile([NCH, NDMA * 128], f32, tag="sel_rep", bufs=1)
        for d in range(NDMA):
            nc.vector.tensor_copy(
                selrep.rearrange("i (d o u) -> i d o u", d=NDMA, u=32)[:, d],
                sel1_sb[:, GCH * d : GCH * (d + 1)].to_broadcast([NCH, GCH, 32]),
            )
        rowidx = []
        for d in range(NDMA):
            rsel_ps = psel.tile([128, 1], f32, space="PSUM", tag="psel")
            nc.tensor.matmul(rsel_ps[:], selrep[:, 128 * d : 128 * (d + 1)], iota64_sb)
            rowf = selp.tile([128, 1], f32, tag="sel_rowf")
            nc.vector.tensor_scalar(
                rowf[:],
                rsel_ps[:],
                32.0,
                iotap32_sb,
                op0=mybir.AluOpType.mult,
                op1=mybir.AluOpType.add,
            )
            ridx = selp.tile([128, 1], i32, tag=f"sel_rowi{d}")
            nc.vector.tensor_copy(ridx[:], rowf[:])
            rowidx.append(ridx)

        # --- cv scatter offsets ---------------------------------------
        # cv viewed [512 rows, 4096]; chunk = 16 rows; scatter of group g
        # writes row 16*dest[8g + p//16] + p%16 (OOB_BIG if dropped).
        comb_sb = selp.tile([NCH, 1], f32, tag="sel_comb")
        nc.vector.tensor_scalar(
            comb_sb[:],
            keep_sb[:],
            -OOB_BIG,
            OOB_BIG,
            op0=mybir.AluOpType.mult,
            op1=mybir.AluOpType.add,
        )
        tmp_sb = selp.tile([NCH, 1], f32, tag="sel_tmp")
        nc.vector.tensor_scalar(
            tmp_sb[:], dest_sb[:], 16.0, None, op0=mybir.AluOpType.mult
        )
        nc.vector.tensor_add(comb_sb[:], comb_sb[:], tmp_sb[:])

        i64rep = selp.tile([NCH, NG * 128], f32, tag="sel_rep", bufs=1)
        for g in range(NG):
            nc.vector.tensor_copy(
                i64rep.rearrange("i (g o u) -> i g o u", g=NG, u=16)[:, g],
                i64_sb[:, CPG * g : CPG * (g + 1)].to_broadcast([NCH, CPG, 16]),
            )
        voffs = []
        for g in range(NG):
            voff_ps = psel.tile([128, 1], f32, space="PSUM", tag="psel")
            nc.tensor.matmul(voff_ps[:], i64rep[:, 128 * g : 128 * (g + 1)], comb_sb[:])
            vof = selp.tile([128, 1], f32, tag="sel_vof")
            nc.vector.tensor_scalar(
                vof[:], voff_ps[:], iotap16_sb, None, op0=mybir.AluOpType.add
            )
            vofi = selp.tile([128, 1], i32, tag=f"sel_vofi{g}")
            nc.vector.tensor_copy(vofi[:], vof[:])
            voffs.append(vofi)

        # --- output DMA -------------------------------------------------
        # All ck gathers issue first: the cv scatters serialize on the cv
        # WAW chain (each waits for the previous one's completion on the
        # GpSimd queue) and would otherwise stall the gather issues too.
        cv_rows = cv.rearrange("(r q) d -> r (q d)", q=4)  # [512, 4096]
        for d in range(NDMA):
            gt = kcp.tile([128, 2048], f32, tag="kc", name=f"gt{d}")
            nc.gpsimd.indirect_dma_start(
                out=gt[:],
                out_offset=None,
                in_=keys_rows,
                in_offset=bass.IndirectOffsetOnAxis(ap=rowidx[d][:], axis=0),
            )
            nc.sync.dma_start(ck_rows[128 * d : 128 * (d + 1), :], gt[:])
        for g in range(NG):
            nc.gpsimd.indirect_dma_start(
                out=cv_rows,
                out_offset=bass.IndirectOffsetOnAxis(ap=voffs[g][:], axis=0),
                in_=vtiles[g].rearrange("p s d -> p (s d)"),
                in_offset=None,
                bounds_check=KEEP * L // 4 - 1,
                oob_is_err=False,
            )

    nc.compile()
    return nc


def _host_consts(W1, b1, W2, mode=MM_MODE):
    f32 = np.float32
    c = np.zeros((128, C_COLS), dtype=f32)
    r64 = np.arange(NCH)
    c[:NCH, C_JLT : C_JLT + NCH] = r64[:, None] < r64[None, :]
    c[:NCH, C_TBM : C_TBM + NCH] = r64[None, :] < r64[:, None]
    c[:NCH, C_I64 : C_I64 + NCH] = np.eye(NCH)
    c[:NCH, C_IOTA32 : C_IOTA32 + KEEP] = np.arange(KEEP)[None, :]
    c[:NCH, C_IOTA64] = r64
    c[:, C_IOTAP32] = np.arange(128) % 32
    c[:, C_IOTAP16] = np.arange(128) % 16
    c[:, C_B1 : C_B1 + NHB] = np.asarray(b1, f32).reshape(NHB, 128).T
    c[:, C_W2 : C_W2 + NHB] = np.asarray(W2, f32)[:, 0].reshape(NHB, 128).T
    c[0, C_ONES : C_ONES + 128] = 1.0
    return {
        "w1": np.ascontiguousarray(W1, dtype=f32),
        "ident": np.eye(128, dtype=f32),
        "consts": c,
    }


def get_nc(mode=MM_MODE):
    key = ("nc", mode)
    if key not in _CACHE:
        _CACHE[key] = _build(mode)
    return _CACHE[key]


def kernel(keys, values, W1, b1, W2, b2):
    from concourse.bass_utils import run_bass_kernel_spmd

    nc = get_nc()
    keys = np.asarray(keys)
    values = np.asarray(values)
    consts = _host_consts(np.asarray(W1), np.asarray(b1), np.asarray(W2))
    in_maps = [dict(keys=keys[i], values=values[i], **consts) for i in range(B)]
    res = run_bass_kernel_spmd(nc, in_maps, list(range(B)))
    ck = np.stack([res.results[i]["ck"] for i in range(B)])
    cv = np.stack([res.results[i]["cv"] for i in range(B)])
    return ck, cv


# revision 47
# speedup vs baseline: 1.3313x; 1.0702x over previous
"""ChunkKVCompressor Trainium2 kernel.

Data-parallel over batch: core i handles batch element i (B=8 across 8 cores).
Per core:
  1. keys/values stream in 8 groups of 512 tokens. values tiles stay RESIDENT
     in SBUF (16 MB) in a quad layout (partition p holds tokens 4p..4p+3 of
     its group) so compressed values scatter straight from SBUF.
     keys load in two matching pair-layout half tiles per group.
  2. c = k + v in-place into the k tiles (DVE), rounded to float32r.
  3. Per d-block: PE transposes (4 blocks) then immediately 4 h-block
     float32r matmuls (N=512, LDWEIGHTS hidden) accumulating into 4 PSUM
     banks -- fine interleave keeps the PE HAM clock warm.
  4. relu(0.5*x + b1) on ACT; chunk sums via two DVE segment reduces
     (token order inside cT is 4*(col%128) + col//128); scores += W2.T@sums
     into one persistent PSUM row [1, 64] (mean/b2 dropped: order-preserving).
  5. On-device top-32-of-64 by rank (comparison matrix + tiny matmuls).
  6. ck: 8x 1MB indirect gathers (one row index per partition, source viewed
     as [2048 rows x 2048 elems]) staged through SBUF + contiguous writes.
     cv: 8x 2MB indirect scatters straight from the resident value tiles
     (cv viewed as [512 rows x 4096 elems]); dropped chunks get row 8192
     and are skipped via bounds_check.
"""

import sys

if "/opt/trn_rl_repo" not in sys.path:
    sys.path.insert(0, "/opt/trn_rl_repo")

import numpy as np
from contextlib import ExitStack

B, T, D, H = 8, 4096, 1024, 512
L = 64  # chunk length (tokens)
NCH = T // L  # 64 chunks
KEEP = 32  # chunks kept per batch
NG = 8  # token groups
GT = T // NG  # 512 tokens per group
NJ = D // 128  # 8 d-blocks
NHB = H // 128  # 4 h-blocks
CPG = NCH // NG  # 8 chunks per group
GCH = 4  # chunks per k-gather DMA
NDMA = KEEP // GCH  # 8 k-gather DMAs
MM_MODE = "f32r"  # "f32r" | "f32"
OOB_BIG = 8192.0  # scatter row for dropped chunks (valid rows 0..511)

# packed fp32 const tensor column layout [128, 384]
C_JLT = 0
C_TBM = 64
C_I64 = 128
C_IOTA32 = 192
C_IOTA64 = 224
C_IOTAP32 = 225
C_IOTAP16 = 226
C_B1 = 229
C_W2 = 233
C_ONES = 240  # [0:1, 240:368]
C_COLS = 384

_CACHE = {}


def _build(mode=MM_MODE):
    import concourse.bass as bass
    import concourse.tile as tile
    from concourse import bacc, mybir

    f32 = mybir.dt.float32
    i32 = mybir.dt.int32
    mm_dt = {"f32r": mybir.dt.float32r, "f32": f32}[mode]

    nc = bacc.Bacc("TRN2", target_bir_lowering=False, debug=False, num_devices=B)

    keys = nc.dram_tensor("keys", [T, D], f32, kind="ExternalInput").ap()
    values = nc.dram_tensor("values", [T, D], f32, kind="ExternalInput").ap()
    w1 = nc.dram_tensor("w1", [D, H], mm_dt, kind="ExternalInput").ap()
    ident_d = nc.dram_tensor("ident", [128, 128], f32, kind="ExternalInput").ap()
    consts_d = nc.dram_tensor("consts", [128, C_COLS], f32, kind="ExternalInput").ap()
    ck = nc.dram_tensor("ck", [KEEP * L, D], f32, kind="ExternalOutput").ap()
    cv = nc.dram_tensor("cv", [KEEP * L, D], f32, kind="ExternalOutput").ap()

    def cast(ap):
        return ap.bitcast(mm_dt) if mode == "f32r" else ap

    with tile.TileContext(nc) as tc, ExitStack() as ctx:
        wp = ctx.enter_context(tc.tile_pool(name="wp", bufs=1))
        kcp = ctx.enter_context(tc.tile_pool(name="kcp", bufs=4))
        vp = ctx.enter_context(tc.tile_pool(name="vp", bufs=NG))
        ctp = ctx.enter_context(tc.tile_pool(name="ctp", bufs=2))
        htp = ctx.enter_context(tc.tile_pool(name="htp", bufs=1))
        csp = ctx.enter_context(tc.tile_pool(name="csp", bufs=8))
        selp = ctx.enter_context(tc.tile_pool(name="selp", bufs=1))
        pst_p = ctx.enter_context(tc.tile_pool(name="pst", bufs=2, space="PSUM"))
        ph_p = ctx.enter_context(tc.tile_pool(name="ph", bufs=1, space="PSUM"))
        sc_p = ctx.enter_context(tc.tile_pool(name="sc", bufs=1, space="PSUM"))
        psel = ctx.enter_context(tc.tile_pool(name="psel", bufs=1, space="PSUM"))

        # --- constants / weights to SBUF (one packed DMA + ident + w1) --
        consts = wp.tile([128, C_COLS], f32)
        nc.sync.dma_start(consts[:], consts_d[:])
        ident_sb = wp.tile([128, 128], mm_dt)
        nc.sync.dma_start(ident_sb[:], ident_d[:].bitcast(mm_dt))

        jlt_sb = consts[:NCH, C_JLT : C_JLT + NCH]
        tbm_sb = consts[:NCH, C_TBM : C_TBM + NCH]
        i64_sb = consts[:NCH, C_I64 : C_I64 + NCH]
        iota32_sb = consts[:NCH, C_IOTA32 : C_IOTA32 + KEEP]
        iota64_sb = consts[:NCH, C_IOTA64 : C_IOTA64 + 1]
        iotap32_sb = consts[:, C_IOTAP32 : C_IOTAP32 + 1]
        iotap16_sb = consts[:, C_IOTAP16 : C_IOTAP16 + 1]
        b1sb = consts[:, C_B1 : C_B1 + NHB]
        w2sb = consts[:, C_W2 : C_W2 + NHB]
        ones_sb = consts[0:1, C_ONES : C_ONES + 128]

        # token tau = 512g + 4p + 2h + e  (k half tiles, h in {0,1})
        keys_g = keys.rearrange("(g p h e) d -> g h p e d", p=128, h=2, e=2)
        # token tau = 512g + 4p + s      (v quad tiles)
        values_g = values.rearrange("(g p s) d -> g p s d", p=128, s=4)

        # preload first group's k/v before w1 so compute starts immediately
        kt_pre = {}
        vtiles = []
        PRE = 2
        for g in range(PRE):
            for h in range(2):
                kt = kcp.tile([128, 2, D], mm_dt, tag="kc", name=f"kt{g}_{h}")
                nc.sync.dma_start(kt[:], cast(keys_g[g, h]))
                kt_pre[(g, h)] = kt
            vt = vp.tile([128, 4, D], f32, tag="v", name=f"vt{g}")
            nc.sync.dma_start(vt[:], values_g[g])
            vtiles.append(vt)

        w1sb = wp.tile([128, NJ, H], mm_dt)  # [p, j, hh]; d = j*128 + p
        nc.sync.dma_start(w1sb[:], w1.rearrange("(j p) hh -> p j hh", p=128))

        scores_ps = sc_p.tile([1, NCH], f32, space="PSUM")

        # pin the PE to the emitted transpose/matmul alternation: the
        # scheduler otherwise batches matmuls densely and lets transposes
        # trickle, which drops the PE HAM clock to 1.2 GHz half the time
        last_pe = [None]

        def pe(bi):
            return bi

        # --- scoring ----------------------------------------------------
        pending_scores = []
        for g in range(NG):
            if g < PRE:
                khalf = [kt_pre[(g, 0)], kt_pre[(g, 1)]]
                vtile = vtiles[g]
            else:
                khalf = []
                for h in range(2):
                    kt = kcp.tile([128, 2, D], mm_dt, tag="kc", name=f"kt{g}_{h}")
                    nc.sync.dma_start(kt[:], cast(keys_g[g, h]))
                    khalf.append(kt)
                vtile = vp.tile([128, 4, D], f32, tag="v", name=f"vt{g}")
                nc.sync.dma_start(vtile[:], values_g[g])
                vtiles.append(vtile)
            # c = k + v in place; h0 on DVE, h1 on GPSIMD so the two adds
            # run concurrently and neither engine gates the transposes long
            nc.vector.tensor_add(
                khalf[0][:], khalf[0][:], vtile[:, 0:2, :]
            )
            nc.gpsimd.tensor_add(
                khalf[1][:], khalf[1][:], vtile[:, 2:4, :]
            )
            # cT columns: col = 128*(2h+e) + p  ->  token 4p + 2h + e
            # two half tiles (j 0-3 / 4-7) so group g+1 can start while
            # group g's second half is still being consumed
            cTa = ctp.tile([128, NJ // 2, GT], mm_dt, tag="cT", name=f"cTa{g}")
            cTb = ctp.tile([128, NJ // 2, GT], mm_dt, tag="cT", name=f"cTb{g}")

            def cT_slice(j):
                t = cTa if j < NJ // 2 else cTb
                return t[:, j % (NJ // 2), :]
            phs = [
                ph_p.tile([128, GT], f32, tag=f"ph{hb}", name=f"ph{hb}_{g}")
                for hb in range(NHB)
            ]
            def emit_mms_hb(hb):
                for j in range(NJ):
                    pe(
                        nc.tensor.matmul(
                            phs[hb][:],
                            w1sb[:, j, 128 * hb : 128 * (hb + 1)],
                            cT_slice(j),
                            start=(j == 0),
                            stop=(j == NJ - 1),
                        )
                    )

            # transposes run one d-block ahead of the matmuls so the PE
            # never waits on the PSUM->SBUF copy of the block it multiplies;
            # the previous group's tiny score matmuls slot into the gaps
            for j in range(NJ):
                pst = pst_p.tile([128, GT], mm_dt, tag="pst")
                for h in range(2):
                    for e in range(2):
                        s = 2 * h + e
                        pe(
                            nc.tensor.transpose(
                                pst[:, 128 * s : 128 * (s + 1)],
                                khalf[h][:, e, 128 * j : 128 * (j + 1)],
                                ident_sb[:],
                            )
                        )
                if j % 2 == 0:
                    nc.vector.tensor_copy(cT_slice(j), pst[:])
                else:
                    nc.scalar.copy(cT_slice(j), pst[:])
            for hb in range(NHB):
                emit_mms_hb(hb)
                if pending_scores:
                    pe(pending_scores.pop(0)())
            for hb in range(NHB):
                ht = htp.tile([128, GT], f32, tag="ht")
                nc.scalar.activation(
                    ht[:],
                    phs[hb][:],
                    mybir.ActivationFunctionType.Relu,
                    bias=b1sb[:, hb : hb + 1],
                    scale=0.5,
                )
                # chunk of col = (col%128)//16; reduce pp then s
                red1 = csp.tile([128, 32], f32, tag="red1")
                nc.vector.tensor_reduce(
                    red1[:],
                    ht.rearrange("p (s pg pp) -> p s pg pp", s=4, pg=8),
                    axis=mybir.AxisListType.X,
                    op=mybir.AluOpType.add,
                )
                csum = csp.tile([128, CPG], f32, tag="csum")
                nc.vector.tensor_reduce(
                    csum[:],
                    red1.rearrange("p (s pg) -> p pg s", s=4),
                    axis=mybir.AxisListType.X,
                    op=mybir.AluOpType.add,
                )

                def make_score(g=g, hb=hb, csum=csum):
                    return nc.tensor.matmul(
                        scores_ps[0:1, CPG * g : CPG * (g + 1)],
                        w2sb[:, hb : hb + 1],
                        csum[:],
                        start=(hb == 0),
                        stop=(hb == NHB - 1),
                    )

                pending_scores.append(make_score)
        while pending_scores:
            pe(pending_scores.pop(0)())

        # --- top-32 selection (all fp32, exact) ------------------------
        scores_sb = selp.tile([1, NCH], f32, tag="sel_s")
        nc.vector.tensor_copy(scores_sb[:], scores_ps[:])

        sT_ps = psel.tile([NCH, 1], f32, space="PSUM", tag="psel")
        nc.tensor.matmul(sT_ps[:], scores_sb[:], ones_sb[:, 0:1])
        sT_sb = selp.tile([NCH, 1], f32, tag="sel_sT")
        nc.vector.tensor_copy(sT_sb[:], sT_ps[:])

        r_ps = psel.tile([NCH, NCH], f32, space="PSUM", tag="psel")
        nc.tensor.matmul(r_ps[:], ones_sb[:, :NCH], scores_sb[:])
        r_sb = selp.tile([NCH, NCH], f32, tag="sel_r")
        nc.vector.tensor_copy(r_sb[:], r_ps[:])

        g_sb = selp.tile([NCH, NCH], f32, tag="sel_g")
        nc.vector.tensor_scalar(
            g_sb[:], r_sb[:], sT_sb[:], None, op0=mybir.AluOpType.is_gt
        )
        eq_sb = selp.tile([NCH, NCH], f32, tag="sel_eq")
        nc.vector.tensor_scalar(
            eq_sb[:], r_sb[:], sT_sb[:], None, op0=mybir.AluOpType.is_equal
        )
        tie_sb = selp.tile([NCH, NCH], f32, tag="sel_tie")
        nc.vector.tensor_mul(tie_sb[:], eq_sb[:], tbm_sb)
        nc.vector.tensor_add(g_sb[:], g_sb[:], tie_sb[:])
        rank_sb = selp.tile([NCH, 1], f32, tag="sel_rank")
        nc.vector.tensor_reduce(
            rank_sb[:], g_sb[:], axis=mybir.AxisListType.X, op=mybir.AluOpType.add
        )
        keep_sb = selp.tile([NCH, 1], f32, tag="sel_keep")
        nc.vector.tensor_scalar(
            keep_sb[:], rank_sb[:], float(KEEP) - 0.5, None, op0=mybir.AluOpType.is_lt
        )

        dest_ps = psel.tile([NCH, 1], f32, space="PSUM", tag="psel")
        nc.tensor.matmul(dest_ps[:], jlt_sb, keep_sb[:])
        dest_sb = selp.tile([NCH, 1], f32, tag="sel_dest")
        nc.vector.tensor_copy(dest_sb[:], dest_ps[:])

        # M[i, o] = 1 iff chunk i goes to slot o
        sel1_sb = selp.tile([NCH, KEEP], f32, tag="sel_m")
        nc.vector.tensor_scalar(
            sel1_sb[:], iota32_sb, dest_sb[:], None, op0=mybir.AluOpType.is_equal
        )
        nc.vector.tensor_scalar(
            sel1_sb[:], sel1_sb[:], keep_sb[:], None, op0=mybir.AluOpType.mult
        )

        # --- ck: slot -> chunk index row for dynamic DRAM->DRAM copies ---
        idxc_ps = psel.tile([KEEP, 1], f32, space="PSUM", tag="psel")
        nc.tensor.matmul(idxc_ps[:], sel1_sb[:], iota64_sb)
        idxc_sb = selp.tile([KEEP, 1], f32, tag="sel_idxc")
        nc.vector.tensor_copy(idxc_sb[:], idxc_ps[:])
        idxr_ps = psel.tile([1, KEEP], f32, space="PSUM", tag="psel")
        nc.tensor.matmul(idxr_ps[:], idxc_sb[:], i64_sb[:KEEP, :KEEP])
        rowb_f = selp.tile([1, KEEP], f32, tag="sel_rowbf")
        nc.vector.tensor_scalar(
            rowb_f[:], idxr_ps[:], 16.0, None, op0=mybir.AluOpType.mult
        )
        rowb_i = selp.tile([1, KEEP], i32, tag="sel_rowbi")
        nc.vector.tensor_copy(rowb_i[:], rowb_f[:])

        # --- cv scatter offsets ---------------------------------------
        # cv viewed [512 rows, 4096]; chunk = 16 rows; scatter of group g
        # writes row 16*dest[8g + p//16] + p%16 (OOB_BIG if dropped).
        comb_sb = selp.tile([NCH, 1], f32, tag="sel_comb")
        nc.vector.tensor_scalar(
            comb_sb[:],
            keep_sb[:],
            -OOB_BIG,
            OOB_BIG,
            op0=mybir.AluOpType.mult,
            op1=mybir.AluOpType.add,
        )
        tmp_sb = selp.tile([NCH, 1], f32, tag="sel_tmp")
        nc.vector.tensor_scalar(
            tmp_sb[:], dest_sb[:], 16.0, None, op0=mybir.AluOpType.mult
        )
        nc.vector.tensor_add(comb_sb[:], comb_sb[:], tmp_sb[:])

        i64rep = selp.tile([NCH, NG * 128], f32, tag="sel_rep", bufs=1)
        for g in range(NG):
            nc.vector.tensor_copy(
                i64rep.rearrange("i (g o u) -> i g o u", g=NG, u=16)[:, g],
                i64_sb[:, CPG * g : CPG * (g + 1)].to_broadcast([NCH, CPG, 16]),
            )
        voffs = []
        for g in range(NG):
            voff_ps = psel.tile([128, 1], f32, space="PSUM", tag="psel")
            nc.tensor.matmul(voff_ps[:], i64rep[:, 128 * g : 128 * (g + 1)], comb_sb[:])
            vof = selp.tile([128, 1], f32, tag="sel_vof")
            nc.vector.tensor_scalar(
                vof[:], voff_ps[:], iotap16_sb, None, op0=mybir.AluOpType.add
            )
            vofi = selp.tile([128, 1], i32, tag=f"sel_vofi{g}")
            nc.vector.tensor_copy(vofi[:], vof[:])
            voffs.append(vofi)

        # --- output DMA -------------------------------------------------
        # ck: 32 HWDGE DRAM->DRAM chunk copies (256KB each) with register
        # source offsets, split across the sync and scalar queues; runs
        # concurrently with the cv scatters on the (single) SWDGE queue.
        keys_r4 = keys.rearrange("(r q) d -> r (q d)", q=4)  # [1024, 4096]
        ck_r4 = ck.rearrange("(r q) d -> r (q d)", q=4)  # [512, 4096]
        cv_rows = cv.rearrange("(r q) d -> r (q d)", q=4)  # [512, 4096]
        for g in range(NG):
            nc.gpsimd.indirect_dma_start(
                out=cv_rows,
                out_offset=bass.IndirectOffsetOnAxis(ap=voffs[g][:], axis=0),
                in_=vtiles[g].rearrange("p s d -> p (s d)"),
                in_offset=None,
                bounds_check=KEEP * L // 4 - 1,
                oob_is_err=False,
            )
        for o in range(KEEP):
            eng = nc.sync if o % 2 == 0 else nc.scalar
            rv = eng.value_load(rowb_i[0:1, o : o + 1])
            eng.dma_start(
                ck_r4[16 * o : 16 * (o + 1), :], keys_r4[bass.ds(rv, 16), :]
            )

    nc.compile()
    return nc


def _host_consts(W1, b1, W2, mode=MM_MODE):
    f32 = np.float32
    c = np.zeros((128, C_COLS), dtype=f32)
    r64 = np.arange(NCH)
    c[:NCH, C_JLT : C_JLT + NCH] = r64[:, None] < r64[None, :]
    c[:NCH, C_TBM : C_TBM + NCH] = r64[None, :] < r64[:, None]
    c[:NCH, C_I64 : C_I64 + NCH] = np.eye(NCH)
    c[:NCH, C_IOTA32 : C_IOTA32 + KEEP] = np.arange(KEEP)[None, :]
    c[:NCH, C_IOTA64] = r64
    c[:, C_IOTAP32] = np.arange(128) % 32
    c[:, C_IOTAP16] = np.arange(128) % 16
    c[:, C_B1 : C_B1 + NHB] = np.asarray(b1, f32).reshape(NHB, 128).T
    c[:, C_W2 : C_W2 + NHB] = np.asarray(W2, f32)[:, 0].reshape(NHB, 128).T
    c[0, C_ONES : C_ONES + 128] = 1.0
    return {
        "w1": np.ascontiguousarray(W1, dtype=f32),
        "ident": np.eye(128, dtype=f32),
        "consts": c,
    }


def get_nc(mode=MM_MODE):
    key = ("nc", mode)
    if key not in _CACHE:
        _CACHE[key] = _build(mode)
    return _CACHE[key]


def kernel(keys, values, W1, b1, W2, b2):
    from concourse.bass_utils import run_bass_kernel_spmd

    nc = get_nc()
    keys = np.asarray(keys)
    values = np.asarray(values)
    consts = _host_consts(np.asarray(W1), np.asarray(b1), np.asarray(W2))
    in_maps = [dict(keys=keys[i], values=values[i], **consts) for i in range(B)]
    res = run_bass_kernel_spmd(nc, in_maps, list(range(B)))
    ck = np.stack([res.results[i]["ck"] for i in range(B)])
    cv = np.stack([res.results[i]["cv"] for i in range(B)])
    return ck, cv


# revision 48
# speedup vs baseline: 1.4298x; 1.0740x over previous
"""ChunkKVCompressor Trainium2 kernel.

Data-parallel over batch: core i handles batch element i (B=8 across 8 cores).
Per core:
  1. keys/values stream in 8 groups of 512 tokens. values tiles stay RESIDENT
     in SBUF (16 MB) in a quad layout (partition p holds tokens 4p..4p+3 of
     its group) so compressed values scatter straight from SBUF.
     keys load in two matching pair-layout half tiles per group.
  2. c = k + v in-place into the k tiles (DVE), rounded to float32r.
  3. Per d-block: PE transposes (4 blocks) then immediately 4 h-block
     float32r matmuls (N=512, LDWEIGHTS hidden) accumulating into 4 PSUM
     banks -- fine interleave keeps the PE HAM clock warm.
  4. relu(0.5*x + b1) on ACT; chunk sums via two DVE segment reduces
     (token order inside cT is 4*(col%128) + col//128); scores += W2.T@sums
     into one persistent PSUM row [1, 64] (mean/b2 dropped: order-preserving).
  5. On-device top-32-of-64 by rank (comparison matrix + tiny matmuls).
  6. ck: 8x 1MB indirect gathers (one row index per partition, source viewed
     as [2048 rows x 2048 elems]) staged through SBUF + contiguous writes.
     cv: 8x 2MB indirect scatters straight from the resident value tiles
     (cv viewed as [512 rows x 4096 elems]); dropped chunks get row 8192
     and are skipped via bounds_check.
"""

import sys

if "/opt/trn_rl_repo" not in sys.path:
    sys.path.insert(0, "/opt/trn_rl_repo")

import numpy as np
from contextlib import ExitStack

B, T, D, H = 8, 4096, 1024, 512
L = 64  # chunk length (tokens)
NCH = T // L  # 64 chunks
KEEP = 32  # chunks kept per batch
NG = 8  # token groups
GT = T // NG  # 512 tokens per group
NJ = D // 128  # 8 d-blocks
NHB = H // 128  # 4 h-blocks
CPG = NCH // NG  # 8 chunks per group
GCH = 4  # chunks per k-gather DMA
NDMA = KEEP // GCH  # 8 k-gather DMAs
MM_MODE = "f32r"  # "f32r" | "f32"
OOB_BIG = 8192.0  # scatter row for dropped chunks (valid rows 0..511)

# packed fp32 const tensor column layout [128, 384]
C_JLT = 0
C_TBM = 64
C_I64 = 128
C_IOTA32 = 192
C_IOTA64 = 224
C_IOTAP32 = 225
C_IOTAP16 = 226
C_B1 = 229
C_W2 = 233
C_ONES = 240  # [0:1, 240:368]
C_COLS = 384

_CACHE = {}


def _build(mode=MM_MODE):
    import concourse.bass as bass
    import concourse.tile as tile
    from concourse import bacc, mybir

    f32 = mybir.dt.float32
    i32 = mybir.dt.int32
    mm_dt = {"f32r": mybir.dt.float32r, "f32": f32}[mode]

    nc = bacc.Bacc("TRN2", target_bir_lowering=False, debug=False, num_devices=B)

    keys = nc.dram_tensor("keys", [T, D], f32, kind="ExternalInput").ap()
    values = nc.dram_tensor("values", [T, D], f32, kind="ExternalInput").ap()
    w1 = nc.dram_tensor("w1", [D, H], mm_dt, kind="ExternalInput").ap()
    ident_d = nc.dram_tensor("ident", [128, 128], f32, kind="ExternalInput").ap()
    consts_d = nc.dram_tensor("consts", [128, C_COLS], f32, kind="ExternalInput").ap()
    ck = nc.dram_tensor("ck", [KEEP * L, D], f32, kind="ExternalOutput").ap()
    cv = nc.dram_tensor("cv", [KEEP * L, D], f32, kind="ExternalOutput").ap()

    def cast(ap):
        return ap.bitcast(mm_dt) if mode == "f32r" else ap

    with tile.TileContext(nc) as tc, ExitStack() as ctx:
        wp = ctx.enter_context(tc.tile_pool(name="wp", bufs=1))
        kcp = ctx.enter_context(tc.tile_pool(name="kcp", bufs=4))
        vp = ctx.enter_context(tc.tile_pool(name="vp", bufs=NG))
        ctp = ctx.enter_context(tc.tile_pool(name="ctp", bufs=2))
        htp = ctx.enter_context(tc.tile_pool(name="htp", bufs=1))
        csp = ctx.enter_context(tc.tile_pool(name="csp", bufs=8))
        selp = ctx.enter_context(tc.tile_pool(name="selp", bufs=1))
        pst_p = ctx.enter_context(tc.tile_pool(name="pst", bufs=2, space="PSUM"))
        ph_p = ctx.enter_context(tc.tile_pool(name="ph", bufs=1, space="PSUM"))
        sc_p = ctx.enter_context(tc.tile_pool(name="sc", bufs=1, space="PSUM"))
        psel = ctx.enter_context(tc.tile_pool(name="psel", bufs=1, space="PSUM"))

        # --- constants / weights to SBUF (one packed DMA + ident + w1) --
        consts = wp.tile([128, C_COLS], f32)
        nc.sync.dma_start(consts[:], consts_d[:])
        ident_sb = wp.tile([128, 128], mm_dt)
        nc.sync.dma_start(ident_sb[:], ident_d[:].bitcast(mm_dt))

        jlt_sb = consts[:NCH, C_JLT : C_JLT + NCH]
        tbm_sb = consts[:NCH, C_TBM : C_TBM + NCH]
        i64_sb = consts[:NCH, C_I64 : C_I64 + NCH]
        iota32_sb = consts[:NCH, C_IOTA32 : C_IOTA32 + KEEP]
        iota64_sb = consts[:NCH, C_IOTA64 : C_IOTA64 + 1]
        iotap32_sb = consts[:, C_IOTAP32 : C_IOTAP32 + 1]
        iotap16_sb = consts[:, C_IOTAP16 : C_IOTAP16 + 1]
        b1sb = consts[:, C_B1 : C_B1 + NHB]
        w2sb = consts[:, C_W2 : C_W2 + NHB]
        ones_sb = consts[0:1, C_ONES : C_ONES + 128]

        # token tau = 512g + 4p + 2h + e  (k half tiles, h in {0,1})
        keys_g = keys.rearrange("(g p h e) d -> g h p e d", p=128, h=2, e=2)
        # token tau = 512g + 4p + s      (v quad tiles)
        values_g = values.rearrange("(g p s) d -> g p s d", p=128, s=4)

        # preload first group's k/v before w1 so compute starts immediately
        kt_pre = {}
        vtiles = []
        PRE = 2
        for g in range(PRE):
            for h in range(2):
                kt = kcp.tile([128, 2, D], mm_dt, tag="kc", name=f"kt{g}_{h}")
                nc.sync.dma_start(kt[:], cast(keys_g[g, h]))
                kt_pre[(g, h)] = kt
            vt = vp.tile([128, 4, D], f32, tag="v", name=f"vt{g}")
            nc.sync.dma_start(vt[:], values_g[g])
            vtiles.append(vt)

        w1sb = wp.tile([128, NJ, H], mm_dt)  # [p, j, hh]; d = j*128 + p
        nc.sync.dma_start(w1sb[:], w1.rearrange("(j p) hh -> p j hh", p=128))

        scores_ps = sc_p.tile([1, NCH], f32, space="PSUM")

        # pin the PE to the emitted transpose/matmul alternation: the
        # scheduler otherwise batches matmuls densely and lets transposes
        # trickle, which drops the PE HAM clock to 1.2 GHz half the time
        last_pe = [None]

        def pe(bi):
            return bi

        # --- scoring ----------------------------------------------------
        pending_scores = []
        for g in range(NG):
            if g < PRE:
                khalf = [kt_pre[(g, 0)], kt_pre[(g, 1)]]
                vtile = vtiles[g]
            else:
                khalf = []
                for h in range(2):
                    kt = kcp.tile([128, 2, D], mm_dt, tag="kc", name=f"kt{g}_{h}")
                    nc.sync.dma_start(kt[:], cast(keys_g[g, h]))
                    khalf.append(kt)
                vtile = vp.tile([128, 4, D], f32, tag="v", name=f"vt{g}")
                nc.sync.dma_start(vtile[:], values_g[g])
                vtiles.append(vtile)
            # c = k + v in place; h0 on DVE, h1 on GPSIMD so the two adds
            # run concurrently and neither engine gates the transposes long
            nc.vector.tensor_add(
                khalf[0][:], khalf[0][:], vtile[:, 0:2, :]
            )
            nc.gpsimd.tensor_add(
                khalf[1][:], khalf[1][:], vtile[:, 2:4, :]
            )
            # cT columns: col = 128*(2h+e) + p  ->  token 4p + 2h + e
            # two half tiles (j 0-3 / 4-7) so group g+1 can start while
            # group g's second half is still being consumed
            cTa = ctp.tile([128, NJ // 2, GT], mm_dt, tag="cT", name=f"cTa{g}")
            cTb = ctp.tile([128, NJ // 2, GT], mm_dt, tag="cT", name=f"cTb{g}")

            def cT_slice(j):
                t = cTa if j < NJ // 2 else cTb
                return t[:, j % (NJ // 2), :]
            phs = [
                ph_p.tile([128, GT], f32, tag=f"ph{hb}", name=f"ph{hb}_{g}")
                for hb in range(NHB)
            ]
            def emit_mms_hb(hb):
                for j in range(NJ):
                    pe(
                        nc.tensor.matmul(
                            phs[hb][:],
                            w1sb[:, j, 128 * hb : 128 * (hb + 1)],
                            cT_slice(j),
                            start=(j == 0),
                            stop=(j == NJ - 1),
                        )
                    )

            # transposes run one d-block ahead of the matmuls so the PE
            # never waits on the PSUM->SBUF copy of the block it multiplies;
            # the previous group's tiny score matmuls slot into the gaps
            for j in range(NJ):
                pst = pst_p.tile([128, GT], mm_dt, tag="pst")
                for h in range(2):
                    for e in range(2):
                        s = 2 * h + e
                        pe(
                            nc.tensor.transpose(
                                pst[:, 128 * s : 128 * (s + 1)],
                                khalf[h][:, e, 128 * j : 128 * (j + 1)],
                                ident_sb[:],
                            )
                        )
                if j % 2 == 0:
                    nc.vector.tensor_copy(cT_slice(j), pst[:])
                else:
                    nc.scalar.copy(cT_slice(j), pst[:])
            for hb in range(NHB):
                emit_mms_hb(hb)
                if pending_scores:
                    pe(pending_scores.pop(0)())
            for hb in range(NHB):
                ht = htp.tile([128, GT], f32, tag="ht")
                nc.scalar.activation(
                    ht[:],
                    phs[hb][:],
                    mybir.ActivationFunctionType.Relu,
                    bias=b1sb[:, hb : hb + 1],
                    scale=0.5,
                )
                # chunk of col = (col%128)//16; reduce pp then s
                red1 = csp.tile([128, 32], f32, tag="red1")
                nc.vector.tensor_reduce(
                    red1[:],
                    ht.rearrange("p (s pg pp) -> p s pg pp", s=4, pg=8),
                    axis=mybir.AxisListType.X,
                    op=mybir.AluOpType.add,
                )
                csum = csp.tile([128, CPG], f32, tag="csum")
                nc.vector.tensor_reduce(
                    csum[:],
                    red1.rearrange("p (s pg) -> p pg s", s=4),
                    axis=mybir.AxisListType.X,
                    op=mybir.AluOpType.add,
                )

                def make_score(g=g, hb=hb, csum=csum):
                    return nc.tensor.matmul(
                        scores_ps[0:1, CPG * g : CPG * (g + 1)],
                        w2sb[:, hb : hb + 1],
                        csum[:],
                        start=(hb == 0),
                        stop=(hb == NHB - 1),
                    )

                pending_scores.append(make_score)
        while pending_scores:
            pe(pending_scores.pop(0)())

        # --- top-32 selection (all fp32, exact) ------------------------
        scores_sb = selp.tile([1, NCH], f32, tag="sel_s")
        nc.vector.tensor_copy(scores_sb[:], scores_ps[:])

        sT_ps = psel.tile([NCH, 1], f32, space="PSUM", tag="psel")
        nc.tensor.matmul(sT_ps[:], scores_sb[:], ones_sb[:, 0:1])
        sT_sb = selp.tile([NCH, 1], f32, tag="sel_sT")
        nc.vector.tensor_copy(sT_sb[:], sT_ps[:])

        r_ps = psel.tile([NCH, NCH], f32, space="PSUM", tag="psel")
        nc.tensor.matmul(r_ps[:], ones_sb[:, :NCH], scores_sb[:])
        r_sb = selp.tile([NCH, NCH], f32, tag="sel_r")
        nc.vector.tensor_copy(r_sb[:], r_ps[:])

        g_sb = selp.tile([NCH, NCH], f32, tag="sel_g")
        nc.vector.tensor_scalar(
            g_sb[:], r_sb[:], sT_sb[:], None, op0=mybir.AluOpType.is_gt
        )
        eq_sb = selp.tile([NCH, NCH], f32, tag="sel_eq")
        nc.vector.tensor_scalar(
            eq_sb[:], r_sb[:], sT_sb[:], None, op0=mybir.AluOpType.is_equal
        )
        tie_sb = selp.tile([NCH, NCH], f32, tag="sel_tie")
        nc.vector.tensor_mul(tie_sb[:], eq_sb[:], tbm_sb)
        nc.vector.tensor_add(g_sb[:], g_sb[:], tie_sb[:])
        rank_sb = selp.tile([NCH, 1], f32, tag="sel_rank")
        nc.vector.tensor_reduce(
            rank_sb[:], g_sb[:], axis=mybir.AxisListType.X, op=mybir.AluOpType.add
        )
        keep_sb = selp.tile([NCH, 1], f32, tag="sel_keep")
        nc.vector.tensor_scalar(
            keep_sb[:], rank_sb[:], float(KEEP) - 0.5, None, op0=mybir.AluOpType.is_lt
        )

        dest_ps = psel.tile([NCH, 1], f32, space="PSUM", tag="psel")
        nc.tensor.matmul(dest_ps[:], jlt_sb, keep_sb[:])
        dest_sb = selp.tile([NCH, 1], f32, tag="sel_dest")
        nc.vector.tensor_copy(dest_sb[:], dest_ps[:])

        # M[i, o] = 1 iff chunk i goes to slot o
        sel1_sb = selp.tile([NCH, KEEP], f32, tag="sel_m")
        nc.vector.tensor_scalar(
            sel1_sb[:], iota32_sb, dest_sb[:], None, op0=mybir.AluOpType.is_equal
        )
        nc.vector.tensor_scalar(
            sel1_sb[:], sel1_sb[:], keep_sb[:], None, op0=mybir.AluOpType.mult
        )

        # --- ck: slot -> chunk index row for dynamic DRAM->DRAM copies ---
        idxc_ps = psel.tile([KEEP, 1], f32, space="PSUM", tag="psel")
        nc.tensor.matmul(idxc_ps[:], sel1_sb[:], iota64_sb)
        idxc_sb = selp.tile([KEEP, 1], f32, tag="sel_idxc")
        nc.vector.tensor_copy(idxc_sb[:], idxc_ps[:])
        idxr_ps = psel.tile([1, KEEP], f32, space="PSUM", tag="psel")
        nc.tensor.matmul(idxr_ps[:], idxc_sb[:], i64_sb[:KEEP, :KEEP])
        rowb_f = selp.tile([1, KEEP], f32, tag="sel_rowbf")
        nc.vector.tensor_scalar(
            rowb_f[:], idxr_ps[:], 16.0, None, op0=mybir.AluOpType.mult
        )
        rowb_i = selp.tile([1, KEEP], i32, tag="sel_rowbi")
        nc.vector.tensor_copy(rowb_i[:], rowb_f[:])

        # --- cv scatter offsets ---------------------------------------
        # cv viewed [512 rows, 4096]; chunk = 16 rows; scatter of group g
        # writes row 16*dest[8g + p//16] + p%16 (OOB_BIG if dropped).
        comb_sb = selp.tile([NCH, 1], f32, tag="sel_comb")
        nc.vector.tensor_scalar(
            comb_sb[:],
            keep_sb[:],
            -OOB_BIG,
            OOB_BIG,
            op0=mybir.AluOpType.mult,
            op1=mybir.AluOpType.add,
        )
        tmp_sb = selp.tile([NCH, 1], f32, tag="sel_tmp")
        nc.vector.tensor_scalar(
            tmp_sb[:], dest_sb[:], 16.0, None, op0=mybir.AluOpType.mult
        )
        nc.vector.tensor_add(comb_sb[:], comb_sb[:], tmp_sb[:])

        i64rep = selp.tile([NCH, NG * 128], f32, tag="sel_rep", bufs=1)
        for g in range(NG):
            nc.vector.tensor_copy(
                i64rep.rearrange("i (g o u) -> i g o u", g=NG, u=16)[:, g],
                i64_sb[:, CPG * g : CPG * (g + 1)].to_broadcast([NCH, CPG, 16]),
            )
        voffs = []
        for g in range(NG):
            voff_ps = psel.tile([128, 1], f32, space="PSUM", tag="psel")
            nc.tensor.matmul(voff_ps[:], i64rep[:, 128 * g : 128 * (g + 1)], comb_sb[:])
            vof = selp.tile([128, 1], f32, tag="sel_vof")
            nc.vector.tensor_scalar(
                vof[:], voff_ps[:], iotap16_sb, None, op0=mybir.AluOpType.add
            )
            vofi = selp.tile([128, 1], i32, tag=f"sel_vofi{g}")
            nc.vector.tensor_copy(vofi[:], vof[:])
            voffs.append(vofi)

        # --- output DMA -------------------------------------------------
        # ck: 32 HWDGE DRAM->DRAM chunk copies (256KB each) with register
        # source offsets, split across the sync and scalar queues; runs
        # concurrently with the cv scatters on the (single) SWDGE queue.
        keys_r4 = keys.rearrange("(r q) d -> r (q d)", q=4)  # [1024, 4096]
        ck_r4 = ck.rearrange("(r q) d -> r (q d)", q=4)  # [512, 4096]
        cv_rows = cv.rearrange("(r q) d -> r (q d)", q=4)  # [512, 4096]
        cv_sem = nc.alloc_semaphore("cv_scatter")
        with tc.tile_critical(no_gpsimd_drain=False):
            nc.gpsimd.sem_clear(cv_sem)
            for g in range(NG):
                nc.gpsimd.indirect_dma_start(
                    out=cv_rows,
                    out_offset=bass.IndirectOffsetOnAxis(ap=voffs[g][:], axis=0),
                    in_=vtiles[g].rearrange("p s d -> p (s d)"),
                    in_offset=None,
                    bounds_check=KEEP * L // 4 - 1,
                    oob_is_err=False,
                ).then_inc(cv_sem, 16)
            nc.gpsimd.wait_ge(cv_sem, 16 * NG)
        for o in range(KEEP):
            eng = nc.sync if o % 2 == 0 else nc.scalar
            rv = eng.value_load(rowb_i[0:1, o : o + 1])
            eng.dma_start(
                ck_r4[16 * o : 16 * (o + 1), :], keys_r4[bass.ds(rv, 16), :]
            )

    nc.compile()
    return nc


def _host_consts(W1, b1, W2, mode=MM_MODE):
    f32 = np.float32
    c = np.zeros((128, C_COLS), dtype=f32)
    r64 = np.arange(NCH)
    c[:NCH, C_JLT : C_JLT + NCH] = r64[:, None] < r64[None, :]
    c[:NCH, C_TBM : C_TBM + NCH] = r64[None, :] < r64[:, None]
    c[:NCH, C_I64 : C_I64 + NCH] = np.eye(NCH)
    c[:NCH, C_IOTA32 : C_IOTA32 + KEEP] = np.arange(KEEP)[None, :]
    c[:NCH, C_IOTA64] = r64
    c[:, C_IOTAP32] = np.arange(128) % 32
    c[:, C_IOTAP16] = np.arange(128) % 16
    c[:, C_B1 : C_B1 + NHB] = np.asarray(b1, f32).reshape(NHB, 128).T
    c[:, C_W2 : C_W2 + NHB] = np.asarray(W2, f32)[:, 0].reshape(NHB, 128).T
    c[0, C_ONES : C_ONES + 128] = 1.0
    return {
        "w1": np.ascontiguousarray(W1, dtype=f32),
        "ident": np.eye(128, dtype=f32),
        "consts": c,
    }


def get_nc(mode=MM_MODE):
    key = ("nc", mode)
    if key not in _CACHE:
        _CACHE[key] = _build(mode)
    return _CACHE[key]


def kernel(keys, values, W1, b1, W2, b2):
    from concourse.bass_utils import run_bass_kernel_spmd

    nc = get_nc()
    keys = np.asarray(keys)
    values = np.asarray(values)
    consts = _host_consts(np.asarray(W1), np.asarray(b1), np.asarray(W2))
    in_maps = [dict(keys=keys[i], values=values[i], **consts) for i in range(B)]
    res = run_bass_kernel_spmd(nc, in_maps, list(range(B)))
    ck = np.stack([res.results[i]["ck"] for i in range(B)])
    cv = np.stack([res.results[i]["cv"] for i in range(B)])
    return ck, cv
